# revision 35
# baseline (speedup 1.0000x reference)
"""Multi-head self-attention (B=4, S=2048, D=1024, 16 heads x 64) on 8 TRN2
NeuronCores via Bass/Tile.

Sharding: tensor-parallel over heads. Each core owns 2 heads (128 of the 1024
Q/K/V output features, column-parallel) and the matching 128 rows of Wo
(row-parallel). Every core computes a full-shape partial output; the host sums
the 8 partials (the row-parallel all-reduce) after gathering.

Per-core dataflow (matmul operands bf16, accumulation fp32 in PSUM):
  xT[b]   : [D, S] features-on-partitions (host pre-transposed)
  qT/kT/vT: [128, S]  = (x @ W)^T per core, via lhsT=W k-tiles, rhs=xT
  v_aug   : PE-transpose of vT -> v natural [S,64] per head + ones column
  scoresT : [j, i] per j-tile pair; the two heads occupy row-groups
            (0-63 / 64-127) of the PE array and run CONCURRENTLY (bass
            auto-derives 64x128 row tiling from the base partitions),
            one [128,1024] PSUM tile per jt
  exp     : one ACT Exp op per (i-chunk, j-tile) covering both heads
            (scale=1/8, per-partition bias = attention-mask column)
  PV      : lhsT=[v_h | ones] [128 j, 65], rhs=expT half, accumulated over
            j-tiles -> rows 0-63 ctx^T, row 64 = softmax denominator
  praw    : right after an i-chunk's last PV, a single DVE copy moves
            pc[0:65] PSUM -> SBUF f32, releasing the pc PSUM banks
            immediately (the next i-chunk's PV accumulation reuses them
            without waiting on the normalize chain)
  norm    : reciprocal_approx_fast of praw denom row + bf16 GPSIMD
            partition broadcast, multiply -> ctxT [128, S] bf16; deferred
            into the NEXT i-chunk's first jp iteration so the ACT exp
            stream never idles behind it (pure SBUF, no PSUM coupling)
  out     : lhsT=ctxT tile [128,128], rhs=Wo_c [128,512] chunks; DVE
            PSUM->SBUF bf16 convert; DMA bf16 partial to DRAM via the
            GPSIMD hwdge queue (8 independently-rounded bf16 partials cost
            ~0.01% extra error and halve the output DMA bytes)

The emission is software-pipelined: batch b's attention steps are interleaved
with batch b+1's QKV/V-transpose units and batch b-1's output-projection
units, so the (in-order) PE always has independent matmul work while the
ACT-bound softmax stream runs. x chunk DMAs for batch b+1 are all queued at
the START of attn(b) (xin ring of 6 chunks) so a projection chain never
head-of-line blocks the PE on an in-flight DMA; the sync hwdge queue carries
ONLY the x stream, secondary constants ride the Activation queue, outputs
ride the GPSIMD queue.
"""

import numpy as np
import ml_dtypes

import concourse.bass as bass
import concourse.mybir as mybir
import concourse.tile as tile
from concourse import bacc, bass_utils
from concourse.masks import make_identity

F32 = mybir.dt.float32
BF16 = mybir.dt.bfloat16
AF = mybir.ActivationFunctionType
BF = ml_dtypes.bfloat16
ts = bass.ts

B, S, D = 4, 2048, 1024
NH, HD = 16, 64
NCORES = 8
OF = D // NCORES            # 128 out-features per core (2 heads)
NKT = D // 128              # 8 contraction tiles
NJT = S // 128              # 16 key tiles per batch
NICH = S // 512             # 4 query chunks per batch
NTT = S // 128              # 16 token tiles per batch


def build_program():
    nc = bacc.Bacc("TRN2", target_bir_lowering=False, debug=False,
                   num_devices=NCORES)
    xT_d = nc.dram_tensor("xT", [B, D, S], BF16, kind="ExternalInput")
    wq_d = nc.dram_tensor("wq", [D, OF], BF16, kind="ExternalInput")
    wk_d = nc.dram_tensor("wk", [D, OF], BF16, kind="ExternalInput")
    wv_d = nc.dram_tensor("wv", [D, OF], BF16, kind="ExternalInput")
    bq_d = nc.dram_tensor("bq", [OF, 1], F32, kind="ExternalInput")
    bk_d = nc.dram_tensor("bk", [OF, 1], F32, kind="ExternalInput")
    bv_d = nc.dram_tensor("bv", [OF, 1], F32, kind="ExternalInput")
    wo_d = nc.dram_tensor("wo", [OF, D], BF16, kind="ExternalInput")
    mask_d = nc.dram_tensor("mask", [128, B * NJT], F32, kind="ExternalInput")
    out_d = nc.dram_tensor("out", [B * S, D], BF16, kind="ExternalOutput")

    with tile.TileContext(nc) as tc:
        with (
            tc.tile_pool(name="consts", bufs=1) as consts,
            tc.tile_pool(name="xin", bufs=8) as xin,
            tc.tile_pool(name="qkv", bufs=2) as qkv,
            tc.tile_pool(name="attn", bufs=4) as attn,
            tc.tile_pool(name="ctxp", bufs=2) as ctxp,
            tc.tile_pool(name="outp", bufs=6) as outp,
            tc.tile_pool(name="psum", bufs=2, space="PSUM") as psum,
        ):
            # ---------------- constants ----------------
            ident = consts.tile([128, 128], BF16)
            make_identity(nc, ident)
            # PE warm-up: ~40 no-dependency matmuls on the identity keep
            # the PE HAM activity window busy during the initial x DMA
            # wait, so the real projection chains start at the full
            # 2.4 GHz clock instead of the cold 1.2 GHz default
            warm = psum.tile([128, 1024], F32, tag="sc", name="sc")
            for _ in range(40):
                nc.tensor.matmul(warm[:, 0:128], lhsT=ident, rhs=ident,
                                 start=True, stop=True)
            # preload the ACT Exp table during the DMA wait so the first
            # real softmax op doesn't pay the ~1.3us table load
            warm_act = consts.tile([1, 8], F32, name="warm_act")
            nc.scalar.activation(warm_act, ident[0:1, 0:8], AF.Exp)
            # wq rides the SP queue ahead of the x stream; wk/wv/biases go
            # via the Activation hwdge queue so they don't delay x pieces
            w_sb = {}
            for nm, d, eng in (("q", wq_d, nc.sync), ("k", wk_d, nc.scalar),
                               ("v", wv_d, nc.scalar)):
                t = consts.tile([128, NKT, OF], BF16, name=f"w{nm}_sb")
                eng.dma_start(t, d[:, :].rearrange("(k p) f -> p k f", p=128))
                w_sb[nm] = t
            b_sb = {}
            for nm, d in (("q", bq_d), ("k", bk_d), ("v", bv_d)):
                t = consts.tile([OF, 1], F32, name=f"b{nm}_sb")
                nc.scalar.dma_start(t, d[:, :])
                b_sb[nm] = t
            # mask is needed by the first exp; wo only by the first
            # outproj much later. Both DMAs are emitted after the first
            # QKV chunk (see below), mask first.
            wo_sb = consts.tile([OF, D], BF16)
            mask_sb = consts.tile([128, B * NJT], F32)
            ones_bc = consts.tile([1, 64], BF16, name="ones_bc")
            nc.vector.memset(ones_bc, 1.0)

            state = [dict() for _ in range(B)]

            def xdma_units(b):
                """x chunk DMAs for batch b, split into k-tile pieces so a
                projection chain starts as soon as its first piece lands.
                Queued a full batch ahead of the matmuls that consume them
                (xin ring of 6 chunks)."""
                st = state[b]
                st["xt"] = []
                xr = xT_d[b].rearrange("(k p) t -> p k t", p=128)
                for nch in range(NICH):
                    xt = xin.tile([128, NKT, 512], BF16, name="xt")
                    step = 1 if b == 0 else 2
                    for k0 in range(0, NKT, step):
                        nc.sync.dma_start(
                            xt[:, k0:k0 + step],
                            xr[:, k0:k0 + step, ts(nch, 512)])
                    st["xt"].append(xt)
                    yield

            def qkv_units(b):
                """QKV projections + V transpose for batch b (chunk-paired
                so accumulation chains alternate PSUM banks)."""
                st = state[b]
                pT = {nm: qkv.tile([OF, S], BF16, name=f"{nm}T")
                      for nm in ("q", "k", "v")}
                st["pT"] = pT
                va = [qkv.tile([128, NJT, 65], BF16, name=f"v_aug{h}")
                      for h in range(2)]
                st["va"] = va
                for h in range(2):
                    nc.vector.memset(va[h][:, :, 64:65], 1.0)
                def proj(nm, nch, xt):
                    # two matmuls per yield: finer grains slow the ACT exp
                    # stream ~20% (denser LDW/MM traffic contends with the
                    # activation engine's SBUF/PSUM access)
                    ps = psum.tile([128, 512], F32, tag="mm",
                                   name="ps_qkv")
                    for kt in range(NKT):
                        nc.tensor.matmul(
                            ps, lhsT=w_sb[nm][:, kt, :],
                            rhs=xt[:, kt, :],
                            start=(kt == 0), stop=(kt == NKT - 1),
                        )
                        if kt % 2 == 1:
                            yield
                    nc.vector.tensor_scalar_add(
                        pT[nm][:, ts(nch, 512)], ps, b_sb[nm])

                def vtrans(nch):
                    for jt in range(4 * nch, 4 * nch + 4):
                        pvt = psum.tile([128, 128], BF16, tag="mm",
                                        name="pvt")
                        nc.tensor.transpose(
                            pvt, pT["v"][:, ts(jt, 128)], ident)
                        for h in range(2):
                            nc.vector.tensor_copy(
                                va[h][:, jt, 0:64],
                                pvt[:, h * 64:(h + 1) * 64])
                        yield

                # scores for j-group g need only kT chunk g (plus qT ch0),
                # and this batch's attention ich0 runs concurrently with
                # this stream: emit k/v/vt chunk-by-chunk, all q's (needed
                # only by ich1+) last
                yield from proj("k", 0, st["xt"][0])
                yield from proj("q", 0, st["xt"][0])
                yield from proj("v", 0, st["xt"][0])
                yield from vtrans(0)
                for nch in range(1, NICH):
                    xt = st["xt"][nch]
                    yield from proj("k", nch, xt)
                    yield from proj("v", nch, xt)
                    yield from vtrans(nch)
                for nch in range(1, NICH):
                    yield from proj("q", nch, st["xt"][nch])

            pending_den = []  # carried (praw, ctxT, isl) across ichs/batches
            carry_pv = []     # one i-chunk's trailing PVs, flushed after the
                              # NEXT i-chunk's first scores+exp are emitted so
                              # they never head-of-line block those scores

            def flush_carry():
                if not carry_pv:
                    return
                emit_fn, pend, pc, cT, cisl = carry_pv.pop()
                while pend:
                    emit_fn(*pend.pop(0))
                # Evacuate the accumulators (one DVE copy per head):
                # releases the pc PSUM ring for the in-flight i-chunk
                # without waiting on the (deferred) normalize chain.
                praw = [attn.tile([65, 512], F32, name=f"praw{h}",
                                  bufs=3) for h in range(2)]
                for h in range(2):
                    nc.vector.tensor_copy(praw[h], pc[h][0:65, :])
                pending_den.append((praw, cT, cisl))

            def den_units(praw, ctxT, isl):
                """Normalize one i-chunk from its praw SBUF copy: ctx times
                1/denominator. Hoisted into the NEXT i-chunk's first jp
                iteration (right after its first Exp is in flight) so the
                ACT stream never idles behind this DVE/GPSIMD chain. Pure
                SBUF: no PSUM coupling with the PV accumulation."""
                rec16s = []
                for h in range(2):
                    # custom-DVE reciprocal needs a partition-0-aligned
                    # input; stage the denominator row through a [1,512]
                    # tile first
                    den = attn.tile([1, 512], F32, name=f"den{h}")
                    nc.vector.tensor_copy(den, praw[h][64:65, :])
                    rec = attn.tile([1, 512], F32, name=f"rec{h}")
                    nc.vector.reciprocal_approx_fast(rec, den)
                    rec16 = attn.tile([1, 512], BF16, name=f"rec16{h}")
                    nc.vector.tensor_copy(rec16, rec)
                    rec16s.append(rec16)
                    yield
                # both heads' DVE work is queued before the first broadcast
                # so the DVE FIFO never head-of-line blocks on GPSIMD; bf16
                # halves the broadcast bytes (~1us in fp32)
                reps = []
                for h in range(2):
                    rep = attn.tile([64, 512], BF16, name=f"rep{h}")
                    nc.gpsimd.partition_broadcast(rep, rec16s[h])
                    reps.append(rep)
                    yield
                for h in range(2):
                    nc.vector.tensor_mul(
                        ctxT[h * 64:(h + 1) * 64, isl],
                        praw[h][0:64, :], reps[h])
                    yield

            def den_units_pc(pc, ctxT, isl):
                """Normalize the FINAL i-chunk straight from its pc PSUM
                banks (nothing follows, so no praw staging — one less dep
                hop on the kernel-end critical path). Copies and casts ride
                the ACT engine (idle once the exps are done) so the DVE
                queue stays clear for the trailing out-unit evacuations."""
                rec16s = []
                for h in range(2):
                    den = attn.tile([1, 512], F32, name=f"den{h}")
                    nc.scalar.activation(den, pc[h][64:65, :], AF.Copy)
                    rec = attn.tile([1, 512], F32, name=f"rec{h}")
                    nc.vector.reciprocal_approx_fast(rec, den)
                    rec16 = attn.tile([1, 512], BF16, name=f"rec16{h}")
                    nc.scalar.activation(rec16, rec, AF.Copy)
                    rec16s.append(rec16)
                    yield
                # K=1 matmul broadcast (ones^T x rec16) on the now-idle PE:
                # ~0.3us vs two serial ~1us GPSIMD partition_broadcasts.
                # ACT (also idle) stages the PSUM result to SBUF — the DVE
                # multiply can read at most one PSUM operand (pc).
                reps = []
                for h in range(2):
                    rep = psum.tile([64, 512], F32, tag="mm",
                                    name=f"repp{h}")
                    nc.tensor.matmul(rep, lhsT=ones_bc, rhs=rec16s[h],
                                     start=True, stop=True)
                    rep_sb = attn.tile([64, 512], BF16, name=f"repsb{h}")
                    nc.scalar.activation(rep_sb, rep, AF.Copy)
                    reps.append(rep_sb)
                    yield
                for h in range(2):
                    nc.vector.tensor_mul(
                        ctxT[h * 64:(h + 1) * 64, isl],
                        pc[h][0:64, :], reps[h])
                    yield

            def attn_units(b):
                """Attention for batch b (normalize chains carried)."""
                st = state[b]
                qT, kT = st["pT"]["q"], st["pT"]["k"]
                va = st["va"]
                ctxT = ctxp.tile([128, S], BF16, name="ctxT")
                st["ctxT"] = ctxT

                for ich in range(NICH):
                    isl = ts(ich, 512)
                    pc = [psum.tile([128, 512], F32, tag="pc", name=f"pc{h}")
                          for h in range(2)]
                    pend_pv = []

                    def emit_pv(jt, et, pc=pc, va=va):
                        for h in range(2):
                            nc.tensor.matmul(
                                pc[h][0:65, :], lhsT=va[h][:, jt, :],
                                rhs=et[:, ts(h, 512)],
                                start=(jt == 0), stop=(jt == NJT - 1),
                            )

                    for jp in range(0, NJT, 2):
                        # two j-tiles of scores back-to-back: their four
                        # row-group-alternating matmuls keep LDWEIGHTS
                        # pull-ahead unblocked (no K=128 matmul between)
                        scs = []
                        for jt in (jp, jp + 1):
                            sc = psum.tile([128, 1024], F32, tag="sc",
                                           name="sc")
                            for h in range(2):
                                hs = slice(h * 64, (h + 1) * 64)
                                nc.tensor.matmul(
                                    sc[:, ts(h, 512)],
                                    lhsT=kT[hs, ts(jt, 128)], rhs=qT[hs, isl],
                                    start=True, stop=True,
                                )
                            scs.append(sc)
                        for idx, jt in enumerate((jp, jp + 1)):
                            et = attn.tile([128, 1024], BF16, name="et",
                                           bufs=8)
                            col = b * NJT + jt
                            nc.scalar.activation(
                                et, scs[idx], AF.Exp,
                                bias=mask_sb[:, col:col + 1], scale=0.125)
                            pend_pv.append((jt, et))
                            if len(pend_pv) > 2:
                                emit_pv(*pend_pv.pop(0))
                            yield
                            if jp == 0 and idx == 0:
                                flush_carry()
                                if pending_den:
                                    yield from den_units(
                                        *pending_den.pop(0))
                    carry_pv.append((emit_pv, pend_pv, pc, ctxT, isl))

            def final_units():
                """Kernel-end: flush the last i-chunk's PVs and normalize
                straight from PSUM. Run via drain() interleaved with the
                held out-units so their matmuls keep the PE warm while this
                DVE/GPSIMD/ACT chain resolves."""
                emit_fn, pend, pc, cT, cisl = carry_pv.pop()
                while pend:
                    emit_fn(*pend.pop(0))
                yield
                yield from den_units_pc(pc, cT, cisl)
                while pending_den:
                    yield from den_units(*pending_den.pop(0))

            def outproj_units(b):
                """Output projection for batch b. 32 yields."""
                ctxT = state[b]["ctxT"]
                for tt in range(NTT):
                    for oc in range(2):
                        # kernel-end tail: borrow the finished score-PSUM
                        # ring (4 idle banks) so trailing matmuls aren't
                        # paced by the 2-bank mm ring's copy-release rate
                        tg = "sc" if (b == B - 1 and tt >= NTT - 4) else "mm"
                        po = psum.tile([128, 512], F32, tag=tg, name="po")
                        nc.tensor.matmul(
                            po, lhsT=ctxT[:, ts(tt, 128)],
                            rhs=wo_sb[:, ts(oc, 512)],
                            start=True, stop=True,
                        )
                        osb = outp.tile([128, 512], BF16, name="osb")
                        if b == B - 1 and tt >= NTT - 4 and oc == 0:
                            # kernel-end tail: ACT is idle (all Exps done);
                            # route half the PSUM->SBUF copies through it so
                            # the trailing units pipeline 2x
                            nc.scalar.activation(osb, po, AF.Copy)
                        else:
                            nc.vector.tensor_copy(osb, po)
                        nc.gpsimd.dma_start(
                            out_d[b * S + tt * 128: b * S + (tt + 1) * 128,
                                  ts(oc, 512)],
                            osb)
                        yield

            def drain(*weighted):
                """weighted: (gen, stride[, delay]) — advance gen every
                `stride` cycles after `delay` cycles. Run until exhausted."""
                live = []
                for w in weighted:
                    g, s, d = (w + (0,)) if len(w) == 2 else w
                    if g is not None:
                        live.append((g, s, d))
                cyc = 0
                while live:
                    nxt = []
                    for g, s, d in live:
                        if cyc >= d and (cyc - d) % s == 0:
                            try:
                                next(g)
                            except StopIteration:
                                continue
                        nxt.append((g, s, d))
                    live = nxt
                    cyc += 1

            def pull(g, n):
                for _ in range(n):
                    try:
                        next(g)
                    except StopIteration:
                        return False
                return True

            g_attn = [attn_units(b) for b in range(B)]
            g_xdma = [xdma_units(b) for b in range(B)]
            g_qkv = [qkv_units(b) for b in range(B)]
            g_out = [outproj_units(b) for b in range(B)]

            # prologue: queue all of batch 0's x stream, then its first
            # QKV chunk; secondary constants (mask before wo — the first
            # exp needs mask) ride the ACT queue behind wk/wv
            pull(g_xdma[0], 4)
            pull(g_qkv[0], 2)
            nc.scalar.dma_start(mask_sb, mask_d[:, :])
            nc.scalar.dma_start(wo_sb, wo_d[:, :])
            pull(g_qkv[0], 10)
            # attention(0) ich0 units unlock per k-chunk: unit u's scores
            # must be emitted AFTER its k-chunk bias-add (emit-before-write
            # would leave the Tile tracker with no dependency to enforce)
            # and its deferred PV after the matching V-transpose. mins[u]
            # is the earliest legal QKV(0)-yield for unit u (+margin).
            mins = [10, 10, 15, 16, 22, 22, 27, 28,
                    34, 34, 39, 40, 46, 46, 51, 52]
            u = cyc = 0
            while pull(g_qkv[0], 1):
                cyc += 1
                if cyc in (40, 44, 48, 52):
                    pull(g_xdma[1], 1)   # x(1) lands before qkv(1) starts
                while u < len(mins) and 12 + cyc >= mins[u] \
                        and pull(g_attn[0], 1):
                    u += 1
            YPI = 22          # attention yields per i-chunk
            for b in range(B - 1):
                # any x(b+1) DMAs not yet queued go first; qkv(b+1) runs
                # 1:1 against attn(b); out(b-1) is delayed so its PE work
                # covers the late (ACT-bound) stretch of attention; x(b+2)
                # queues late in this iteration so its chunks land well
                # before qkv(b+2) chains issue
                pull(g_xdma[b + 1], 4)
                a, q, o = g_attn[b], g_qkv[b + 1], \
                    (g_out[b - 1] if b >= 1 else None)
                cyc = 0
                while pull(a, 1):
                    cyc += 1
                    pull(q, 1)
                    if o is not None and cyc % 3 == 0:
                        pull(o, 1)
                    if b + 2 < B and cyc in (40, 44, 48, 52):
                        pull(g_xdma[b + 2], 1)
                drain((q, 1), (o, 1) if o is not None else (None, 1))
            # final batch: out(3) units become ready 8 per i-chunk; emit them
            # as soon as ready (never earlier — the in-order PE queue would
            # head-of-line block) alongside the tail of out(2)
            a, o2, o3 = g_attn[B - 1], g_out[B - 2], g_out[B - 1]
            cyc = adv3 = 0
            while pull(a, 1):
                cyc += 1
                if (cyc * 2) % 5 < 2:
                    pull(o2, 1)
                # hold the 8 ich2 units (generator positions 17-24) for the
                # final drain: they are dependency-free by then and their
                # matmuls keep the PE busy (HAM warm) while the last
                # i-chunk's normalize chain (DVE/GPSIMD only) runs
                ready = 8 * max(0, (cyc - 8) // YPI)
                if adv3 < min(ready, 16) and cyc % 2 == 0:
                    if pull(o3, 1):
                        adv3 += 1
            drain((final_units(), 1), (o3, 1))
            drain((o2, 1), (o3, 1))
    nc.finalize()
    return nc


def make_in_maps(x, attention_mask, Wq, bq, Wk, bk, Wv, bv, Wo, bo):
    x = np.asarray(x, dtype=np.float32)
    attention_mask = np.asarray(attention_mask, dtype=np.float32)
    Wq, Wk, Wv, Wo = (np.asarray(a, dtype=np.float32) for a in (Wq, Wk, Wv, Wo))
    bq, bk, bv, bo = (np.asarray(a, dtype=np.float32) for a in (bq, bk, bv, bo))

    xT = np.ascontiguousarray(x.transpose(0, 2, 1)).astype(BF)  # [B, D, S]
    # mask[b,0,0,j] -> [128 partitions, B*NJT] column per (batch, j-tile)
    m = attention_mask.reshape(B, S).reshape(B, NJT, 128)
    mask_host = np.ascontiguousarray(m.transpose(2, 0, 1).reshape(128, B * NJT))

    in_maps = []
    for c in range(NCORES):
        cs = slice(c * OF, (c + 1) * OF)
        in_maps.append({
            "xT": xT,
            "wq": np.ascontiguousarray(Wq[:, cs]).astype(BF),
            "wk": np.ascontiguousarray(Wk[:, cs]).astype(BF),
            "wv": np.ascontiguousarray(Wv[:, cs]).astype(BF),
            "bq": np.ascontiguousarray(bq[cs]).reshape(OF, 1),
            "bk": np.ascontiguousarray(bk[cs]).reshape(OF, 1),
            "bv": np.ascontiguousarray(bv[cs]).reshape(OF, 1),
            "wo": np.ascontiguousarray(Wo[cs, :]).astype(BF),
            "mask": mask_host,
        })
    return in_maps


def combine_outputs(results, bo):
    acc = np.zeros((B * S, D), dtype=np.float64)
    for r in results:
        acc += r["out"].astype(np.float64)
    acc += np.asarray(bo, dtype=np.float64)
    return acc.reshape(B, S, D).astype(np.float32)


_NC_CACHE = []


def _get_program():
    if not _NC_CACHE:
        _NC_CACHE.append(build_program())
    return _NC_CACHE[0]


def kernel(**inputs):
    nc = _get_program()
    in_maps = make_in_maps(**inputs)
    res = bass_utils.run_bass_kernel_spmd(
        nc, in_maps, core_ids=list(range(NCORES)))
    return combine_outputs(res.results, inputs["bo"])


# revision 39
# speedup vs baseline: 1.0014x; 1.0014x over previous
"""Multi-head self-attention (B=4, S=2048, D=1024, 16 heads x 64) on 8 TRN2
NeuronCores via Bass/Tile.

Sharding: tensor-parallel over heads. Each core owns 2 heads (128 of the 1024
Q/K/V output features, column-parallel) and the matching 128 rows of Wo
(row-parallel). Every core computes a full-shape partial output; the host sums
the 8 partials (the row-parallel all-reduce) after gathering.

Per-core dataflow (matmul operands bf16, accumulation fp32 in PSUM):
  xT[b]   : [D, S] features-on-partitions (host pre-transposed)
  qT/kT/vT: [128, S]  = (x @ W)^T per core, via lhsT=W k-tiles, rhs=xT
  v_aug   : PE-transpose of vT -> v natural [S,64] per head + ones column
  scoresT : [j, i] per j-tile pair; the two heads occupy row-groups
            (0-63 / 64-127) of the PE array and run CONCURRENTLY (bass
            auto-derives 64x128 row tiling from the base partitions),
            one [128,1024] PSUM tile per jt
  exp     : one ACT Exp op per (i-chunk, j-tile) covering both heads
            (scale=1/8, per-partition bias = attention-mask column)
  PV      : lhsT=[v_h | ones] [128 j, 65], rhs=expT half, accumulated over
            j-tiles -> rows 0-63 ctx^T, row 64 = softmax denominator
  praw    : right after an i-chunk's last PV, a single DVE copy moves
            pc[0:65] PSUM -> SBUF f32, releasing the pc PSUM banks
            immediately (the next i-chunk's PV accumulation reuses them
            without waiting on the normalize chain)
  norm    : reciprocal_approx_fast of praw denom row + bf16 GPSIMD
            partition broadcast, multiply -> ctxT [128, S] bf16; deferred
            into the NEXT i-chunk's first jp iteration so the ACT exp
            stream never idles behind it (pure SBUF, no PSUM coupling)
  out     : lhsT=ctxT tile [128,128], rhs=Wo_c [128,512] chunks; DVE
            PSUM->SBUF bf16 convert; DMA bf16 partial to DRAM via the
            GPSIMD hwdge queue (8 independently-rounded bf16 partials cost
            ~0.01% extra error and halve the output DMA bytes)

The emission is software-pipelined: batch b's attention steps are interleaved
with batch b+1's QKV/V-transpose units and batch b-1's output-projection
units, so the (in-order) PE always has independent matmul work while the
ACT-bound softmax stream runs. x chunk DMAs for batch b+1 are all queued at
the START of attn(b) (xin ring of 6 chunks) so a projection chain never
head-of-line blocks the PE on an in-flight DMA; the sync hwdge queue carries
ONLY the x stream, secondary constants ride the Activation queue, outputs
ride the GPSIMD queue.
"""

import numpy as np
import ml_dtypes

import concourse.bass as bass
import concourse.mybir as mybir
import concourse.tile as tile
from concourse import bacc, bass_utils
from concourse.masks import make_identity

F32 = mybir.dt.float32
BF16 = mybir.dt.bfloat16
AF = mybir.ActivationFunctionType
BF = ml_dtypes.bfloat16
ts = bass.ts

B, S, D = 4, 2048, 1024
NH, HD = 16, 64
NCORES = 8
OF = D // NCORES            # 128 out-features per core (2 heads)
NKT = D // 128              # 8 contraction tiles
NJT = S // 128              # 16 key tiles per batch
NICH = S // 512             # 4 query chunks per batch
NTT = S // 128              # 16 token tiles per batch


def build_program():
    nc = bacc.Bacc("TRN2", target_bir_lowering=False, debug=False,
                   num_devices=NCORES)
    xT_d = nc.dram_tensor("xT", [B, D, S], BF16, kind="ExternalInput")
    wq_d = nc.dram_tensor("wq", [D, OF], BF16, kind="ExternalInput")
    wk_d = nc.dram_tensor("wk", [D, OF], BF16, kind="ExternalInput")
    wv_d = nc.dram_tensor("wv", [D, OF], BF16, kind="ExternalInput")
    bq_d = nc.dram_tensor("bq", [OF, 1], F32, kind="ExternalInput")
    bk_d = nc.dram_tensor("bk", [OF, 1], F32, kind="ExternalInput")
    bv_d = nc.dram_tensor("bv", [OF, 1], F32, kind="ExternalInput")
    wo_d = nc.dram_tensor("wo", [OF, D], BF16, kind="ExternalInput")
    mask_d = nc.dram_tensor("mask", [128, B * NJT], F32, kind="ExternalInput")
    out_d = nc.dram_tensor("out", [B * S, D], BF16, kind="ExternalOutput")

    with tile.TileContext(nc) as tc:
        with (
            tc.tile_pool(name="consts", bufs=1) as consts,
            tc.tile_pool(name="xin", bufs=8) as xin,
            tc.tile_pool(name="qkv", bufs=2) as qkv,
            tc.tile_pool(name="attn", bufs=4) as attn,
            tc.tile_pool(name="ctxp", bufs=2) as ctxp,
            tc.tile_pool(name="outp", bufs=6) as outp,
            tc.tile_pool(name="psum", bufs=2, space="PSUM") as psum,
        ):
            # ---------------- constants ----------------
            ident = consts.tile([128, 128], BF16)
            make_identity(nc, ident)
            # PE warm-up: ~40 no-dependency matmuls on the identity keep
            # the PE HAM activity window busy during the initial x DMA
            # wait, so the real projection chains start at the full
            # 2.4 GHz clock instead of the cold 1.2 GHz default
            warm = psum.tile([128, 1024], F32, tag="sc", name="sc")
            for _ in range(40):
                nc.tensor.matmul(warm[:, 0:128], lhsT=ident, rhs=ident,
                                 start=True, stop=True)
            # preload the ACT Exp table during the DMA wait so the first
            # real softmax op doesn't pay the ~1.3us table load
            warm_act = consts.tile([1, 8], F32, name="warm_act")
            nc.scalar.activation(warm_act, ident[0:1, 0:8], AF.Exp)
            # wq rides the SP queue ahead of the x stream; wk/wv/biases go
            # via the Activation hwdge queue so they don't delay x pieces
            w_sb = {}
            for nm, d, eng in (("q", wq_d, nc.sync), ("k", wk_d, nc.scalar),
                               ("v", wv_d, nc.scalar)):
                t = consts.tile([128, NKT, OF], BF16, name=f"w{nm}_sb")
                eng.dma_start(t, d[:, :].rearrange("(k p) f -> p k f", p=128))
                w_sb[nm] = t
            b_sb = {}
            for nm, d in (("q", bq_d), ("k", bk_d), ("v", bv_d)):
                t = consts.tile([OF, 1], F32, name=f"b{nm}_sb")
                nc.scalar.dma_start(t, d[:, :])
                b_sb[nm] = t
            # mask is needed by the first exp; wo only by the first
            # outproj much later. Both DMAs are emitted after the first
            # QKV chunk (see below), mask first.
            wo_sb = consts.tile([OF, D], BF16)
            mask_sb = consts.tile([128, B * NJT], F32)
            ones_bc = consts.tile([1, 64], BF16, name="ones_bc")
            nc.vector.memset(ones_bc, 1.0)

            state = [dict() for _ in range(B)]

            def xdma_units(b):
                """x chunk DMAs for batch b, split into k-tile pieces so a
                projection chain starts as soon as its first piece lands.
                Queued a full batch ahead of the matmuls that consume them
                (xin ring of 6 chunks)."""
                st = state[b]
                st["xt"] = []
                xr = xT_d[b].rearrange("(k p) t -> p k t", p=128)
                for nch in range(NICH):
                    xt = xin.tile([128, NKT, 512], BF16, name="xt")
                    step = 1 if b == 0 else 2
                    for k0 in range(0, NKT, step):
                        nc.sync.dma_start(
                            xt[:, k0:k0 + step],
                            xr[:, k0:k0 + step, ts(nch, 512)])
                    st["xt"].append(xt)
                    yield

            def qkv_units(b):
                """QKV projections + V transpose for batch b (chunk-paired
                so accumulation chains alternate PSUM banks)."""
                st = state[b]
                pT = {nm: qkv.tile([OF, S], BF16, name=f"{nm}T")
                      for nm in ("q", "k", "v")}
                st["pT"] = pT
                va = [qkv.tile([128, NJT, 65], BF16, name=f"v_aug{h}")
                      for h in range(2)]
                st["va"] = va
                for h in range(2):
                    nc.vector.memset(va[h][:, :, 64:65], 1.0)
                def proj(nm, nch, xt):
                    # two matmuls per yield: finer grains slow the ACT exp
                    # stream ~20% (denser LDW/MM traffic contends with the
                    # activation engine's SBUF/PSUM access)
                    ps = psum.tile([128, 512], F32, tag="mm",
                                   name="ps_qkv")
                    for kt in range(NKT):
                        nc.tensor.matmul(
                            ps, lhsT=w_sb[nm][:, kt, :],
                            rhs=xt[:, kt, :],
                            start=(kt == 0), stop=(kt == NKT - 1),
                        )
                        if kt % 2 == 1:
                            yield
                    nc.vector.tensor_scalar_add(
                        pT[nm][:, ts(nch, 512)], ps, b_sb[nm])

                def vtrans(nch):
                    for jt in range(4 * nch, 4 * nch + 4):
                        pvt = psum.tile([128, 128], BF16, tag="mm",
                                        name="pvt")
                        nc.tensor.transpose(
                            pvt, pT["v"][:, ts(jt, 128)], ident)
                        for h in range(2):
                            nc.vector.tensor_copy(
                                va[h][:, jt, 0:64],
                                pvt[:, h * 64:(h + 1) * 64])
                        yield

                # scores for j-group g need only kT chunk g (plus qT's own
                # chunk), and this batch's attention ich0 runs concurrently
                # with this stream: k/v/vt lead each chunk; q1 follows
                # chunk 1 (ich1 needs it soon after ich0), q2/q3 trail
                yield from proj("k", 0, st["xt"][0])
                yield from proj("q", 0, st["xt"][0])
                yield from proj("v", 0, st["xt"][0])
                yield from vtrans(0)
                yield from proj("k", 1, st["xt"][1])
                yield from proj("v", 1, st["xt"][1])
                yield from vtrans(1)
                yield from proj("q", 1, st["xt"][1])
                for nch in (2, 3):
                    xt = st["xt"][nch]
                    yield from proj("k", nch, xt)
                    yield from proj("v", nch, xt)
                    yield from vtrans(nch)
                yield from proj("q", 2, st["xt"][2])
                yield from proj("q", 3, st["xt"][3])

            pending_den = []  # carried (praw, ctxT, isl) across ichs/batches
            carry_pv = []     # one i-chunk's trailing PVs, flushed after the
                              # NEXT i-chunk's first scores+exp are emitted so
                              # they never head-of-line block those scores

            def flush_carry():
                if not carry_pv:
                    return
                emit_fn, pend, pc, cT, cisl = carry_pv.pop()
                while pend:
                    emit_fn(*pend.pop(0))
                # Evacuate the accumulators (one DVE copy per head):
                # releases the pc PSUM ring for the in-flight i-chunk
                # without waiting on the (deferred) normalize chain.
                praw = [attn.tile([65, 512], F32, name=f"praw{h}",
                                  bufs=3) for h in range(2)]
                for h in range(2):
                    nc.vector.tensor_copy(praw[h], pc[h][0:65, :])
                pending_den.append((praw, cT, cisl))

            def den_units(praw, ctxT, isl):
                """Normalize one i-chunk from its praw SBUF copy: ctx times
                1/denominator. Hoisted into the NEXT i-chunk's first jp
                iteration (right after its first Exp is in flight) so the
                ACT stream never idles behind this DVE/GPSIMD chain. Pure
                SBUF: no PSUM coupling with the PV accumulation."""
                rec16s = []
                for h in range(2):
                    # custom-DVE reciprocal needs a partition-0-aligned
                    # input; stage the denominator row through a [1,512]
                    # tile first
                    den = attn.tile([1, 512], F32, name=f"den{h}")
                    nc.vector.tensor_copy(den, praw[h][64:65, :])
                    rec = attn.tile([1, 512], F32, name=f"rec{h}")
                    nc.vector.reciprocal_approx_fast(rec, den)
                    rec16 = attn.tile([1, 512], BF16, name=f"rec16{h}")
                    nc.vector.tensor_copy(rec16, rec)
                    rec16s.append(rec16)
                    yield
                # both heads' DVE work is queued before the first broadcast
                # so the DVE FIFO never head-of-line blocks on GPSIMD; bf16
                # halves the broadcast bytes (~1us in fp32)
                reps = []
                for h in range(2):
                    rep = attn.tile([64, 512], BF16, name=f"rep{h}")
                    nc.gpsimd.partition_broadcast(rep, rec16s[h])
                    reps.append(rep)
                    yield
                for h in range(2):
                    nc.vector.tensor_mul(
                        ctxT[h * 64:(h + 1) * 64, isl],
                        praw[h][0:64, :], reps[h])
                    yield

            def den_units_pc(pc, ctxT, isl):
                """Normalize the FINAL i-chunk straight from its pc PSUM
                banks (nothing follows, so no praw staging — one less dep
                hop on the kernel-end critical path). Copies and casts ride
                the ACT engine (idle once the exps are done) so the DVE
                queue stays clear for the trailing out-unit evacuations."""
                rec16s = []
                for h in range(2):
                    den = attn.tile([1, 512], F32, name=f"den{h}")
                    nc.scalar.activation(den, pc[h][64:65, :], AF.Copy)
                    rec = attn.tile([1, 512], F32, name=f"rec{h}")
                    nc.vector.reciprocal_approx_fast(rec, den)
                    rec16 = attn.tile([1, 512], BF16, name=f"rec16{h}")
                    nc.vector.tensor_copy(rec16, rec)
                    rec16s.append(rec16)
                    yield
                # K=1 matmul broadcast (ones^T x rec16) on the now-idle PE:
                # ~0.3us vs two serial ~1us GPSIMD partition_broadcasts.
                # ACT (also idle) stages the PSUM result to SBUF — the DVE
                # multiply can read at most one PSUM operand (pc).
                reps = []
                for h in range(2):
                    rep = psum.tile([64, 512], F32, tag="mm",
                                    name=f"repp{h}")
                    nc.tensor.matmul(rep, lhsT=ones_bc, rhs=rec16s[h],
                                     start=True, stop=True)
                    rep_sb = attn.tile([64, 512], BF16, name=f"repsb{h}")
                    nc.scalar.activation(rep_sb, rep, AF.Copy)
                    reps.append(rep_sb)
                    yield
                for h in range(2):
                    nc.vector.tensor_mul(
                        ctxT[h * 64:(h + 1) * 64, isl],
                        pc[h][0:64, :], reps[h])
                    yield

            def attn_units(b):
                """Attention for batch b (normalize chains carried)."""
                st = state[b]
                qT, kT = st["pT"]["q"], st["pT"]["k"]
                va = st["va"]
                ctxT = ctxp.tile([128, S], BF16, name="ctxT")
                st["ctxT"] = ctxT

                for ich in range(NICH):
                    isl = ts(ich, 512)
                    pc = [psum.tile([128, 512], F32, tag="pc", name=f"pc{h}")
                          for h in range(2)]
                    pend_pv = []

                    def emit_pv(jt, et, pc=pc, va=va):
                        for h in range(2):
                            nc.tensor.matmul(
                                pc[h][0:65, :], lhsT=va[h][:, jt, :],
                                rhs=et[:, ts(h, 512)],
                                start=(jt == 0), stop=(jt == NJT - 1),
                            )

                    for jp in range(0, NJT, 2):
                        # two j-tiles of scores back-to-back: their four
                        # row-group-alternating matmuls keep LDWEIGHTS
                        # pull-ahead unblocked (no K=128 matmul between)
                        scs = []
                        for jt in (jp, jp + 1):
                            sc = psum.tile([128, 1024], F32, tag="sc",
                                           name="sc")
                            for h in range(2):
                                hs = slice(h * 64, (h + 1) * 64)
                                nc.tensor.matmul(
                                    sc[:, ts(h, 512)],
                                    lhsT=kT[hs, ts(jt, 128)], rhs=qT[hs, isl],
                                    start=True, stop=True,
                                )
                            scs.append(sc)
                        for idx, jt in enumerate((jp, jp + 1)):
                            et = attn.tile([128, 1024], BF16, name="et",
                                           bufs=8)
                            col = b * NJT + jt
                            nc.scalar.activation(
                                et, scs[idx], AF.Exp,
                                bias=mask_sb[:, col:col + 1], scale=0.125)
                            pend_pv.append((jt, et))
                            if len(pend_pv) > 2:
                                emit_pv(*pend_pv.pop(0))
                            yield
                            if jp == 0 and idx == 0:
                                flush_carry()
                                if pending_den:
                                    yield from den_units(
                                        *pending_den.pop(0))
                    carry_pv.append((emit_pv, pend_pv, pc, ctxT, isl))

            def final_units():
                """Kernel-end: flush the last i-chunk's PVs and normalize
                straight from PSUM. Run via drain() interleaved with the
                held out-units so their matmuls keep the PE warm while this
                DVE/GPSIMD/ACT chain resolves."""
                emit_fn, pend, pc, cT, cisl = carry_pv.pop()
                while pend:
                    emit_fn(*pend.pop(0))
                yield
                yield from den_units_pc(pc, cT, cisl)
                while pending_den:
                    yield from den_units(*pending_den.pop(0))

            def outproj_units(b):
                """Output projection for batch b. 32 yields."""
                ctxT = state[b]["ctxT"]
                for tt in range(NTT):
                    for oc in range(2):
                        # kernel-end tail: borrow the finished score-PSUM
                        # ring (4 idle banks) so trailing matmuls aren't
                        # paced by the 2-bank mm ring's copy-release rate
                        tg = "sc" if (b == B - 1 and tt >= NTT - 4) else "mm"
                        po = psum.tile([128, 512], F32, tag=tg, name="po")
                        nc.tensor.matmul(
                            po, lhsT=ctxT[:, ts(tt, 128)],
                            rhs=wo_sb[:, ts(oc, 512)],
                            start=True, stop=True,
                        )
                        osb = outp.tile([128, 512], BF16, name="osb")
                        if b == B - 1 and tt >= NTT - 4 and oc == 0:
                            # kernel-end tail: ACT is idle (all Exps done);
                            # route half the PSUM->SBUF copies through it so
                            # the trailing units pipeline 2x
                            nc.scalar.activation(osb, po, AF.Copy)
                        else:
                            nc.vector.tensor_copy(osb, po)
                        # alternate hwdge queues: a single queue issues one
                        # ~650ns trigger at a time and serializes the
                        # kernel-end DMA drain
                        eng = nc.gpsimd if oc == 0 else nc.sync
                        eng.dma_start(
                            out_d[b * S + tt * 128: b * S + (tt + 1) * 128,
                                  ts(oc, 512)],
                            osb)
                        yield

            def drain(*weighted):
                """weighted: (gen, stride[, delay]) — advance gen every
                `stride` cycles after `delay` cycles. Run until exhausted."""
                live = []
                for w in weighted:
                    g, s, d = (w + (0,)) if len(w) == 2 else w
                    if g is not None:
                        live.append((g, s, d))
                cyc = 0
                while live:
                    nxt = []
                    for g, s, d in live:
                        if cyc >= d and (cyc - d) % s == 0:
                            try:
                                next(g)
                            except StopIteration:
                                continue
                        nxt.append((g, s, d))
                    live = nxt
                    cyc += 1

            def pull(g, n):
                for _ in range(n):
                    try:
                        next(g)
                    except StopIteration:
                        return False
                return True

            g_attn = [attn_units(b) for b in range(B)]
            g_xdma = [xdma_units(b) for b in range(B)]
            g_qkv = [qkv_units(b) for b in range(B)]
            g_out = [outproj_units(b) for b in range(B)]

            # prologue: queue all of batch 0's x stream, then its first
            # QKV chunk; secondary constants (mask before wo — the first
            # exp needs mask) ride the ACT queue behind wk/wv
            pull(g_xdma[0], 4)
            pull(g_qkv[0], 2)
            nc.scalar.dma_start(mask_sb, mask_d[:, :])
            nc.scalar.dma_start(wo_sb, wo_d[:, :])
            pull(g_qkv[0], 10)
            # attention(0) ich0 units unlock per k-chunk: unit u's scores
            # must be emitted AFTER its k-chunk bias-add (emit-before-write
            # would leave the Tile tracker with no dependency to enforce)
            # and its deferred PV after the matching V-transpose. mins[u]
            # is the earliest legal QKV(0)-yield for unit u (+margin).
            mins = [10, 10, 15, 16, 22, 22, 27, 28,
                    38, 38, 43, 44, 50, 50, 55, 56]
            u = cyc = 0
            while pull(g_qkv[0], 1):
                cyc += 1
                if cyc in (40, 44, 48, 52):
                    pull(g_xdma[1], 1)   # x(1) lands before qkv(1) starts
                while u < len(mins) and 12 + cyc >= mins[u] \
                        and pull(g_attn[0], 1):
                    u += 1
            YPI = 22          # attention yields per i-chunk
            for b in range(B - 1):
                # any x(b+1) DMAs not yet queued go first; qkv(b+1) runs
                # 1:1 against attn(b); out(b-1) is delayed so its PE work
                # covers the late (ACT-bound) stretch of attention; x(b+2)
                # queues late in this iteration so its chunks land well
                # before qkv(b+2) chains issue
                pull(g_xdma[b + 1], 4)
                a, q, o = g_attn[b], g_qkv[b + 1], \
                    (g_out[b - 1] if b >= 1 else None)
                cyc = 0
                while pull(a, 1):
                    cyc += 1
                    pull(q, 1)
                    if o is not None and cyc % 3 == 0:
                        pull(o, 1)
                    if b + 2 < B and cyc in (40, 44, 48, 52):
                        pull(g_xdma[b + 2], 1)
                drain((q, 1), (o, 1) if o is not None else (None, 1))
            # final batch: out(3) units become ready 8 per i-chunk; emit them
            # as soon as ready (never earlier — the in-order PE queue would
            # head-of-line block) alongside the tail of out(2)
            a, o2, o3 = g_attn[B - 1], g_out[B - 2], g_out[B - 1]
            cyc = adv3 = 0
            while pull(a, 1):
                cyc += 1
                if (cyc * 2) % 5 < 2:
                    pull(o2, 1)
                # hold the 8 ich2 units (generator positions 17-24) for the
                # final drain: they are dependency-free by then and their
                # matmuls keep the PE busy (HAM warm) while the last
                # i-chunk's normalize chain (DVE/GPSIMD only) runs
                ready = 8 * max(0, (cyc - 8) // YPI)
                if adv3 < min(ready, 16) and cyc % 2 == 0:
                    if pull(o3, 1):
                        adv3 += 1
            drain((final_units(), 1), (o3, 1))
            drain((o2, 1), (o3, 1))
    nc.finalize()
    return nc


def make_in_maps(x, attention_mask, Wq, bq, Wk, bk, Wv, bv, Wo, bo):
    x = np.asarray(x, dtype=np.float32)
    attention_mask = np.asarray(attention_mask, dtype=np.float32)
    Wq, Wk, Wv, Wo = (np.asarray(a, dtype=np.float32) for a in (Wq, Wk, Wv, Wo))
    bq, bk, bv, bo = (np.asarray(a, dtype=np.float32) for a in (bq, bk, bv, bo))

    xT = np.ascontiguousarray(x.transpose(0, 2, 1)).astype(BF)  # [B, D, S]
    # mask[b,0,0,j] -> [128 partitions, B*NJT] column per (batch, j-tile)
    m = attention_mask.reshape(B, S).reshape(B, NJT, 128)
    mask_host = np.ascontiguousarray(m.transpose(2, 0, 1).reshape(128, B * NJT))

    in_maps = []
    for c in range(NCORES):
        cs = slice(c * OF, (c + 1) * OF)
        in_maps.append({
            "xT": xT,
            "wq": np.ascontiguousarray(Wq[:, cs]).astype(BF),
            "wk": np.ascontiguousarray(Wk[:, cs]).astype(BF),
            "wv": np.ascontiguousarray(Wv[:, cs]).astype(BF),
            "bq": np.ascontiguousarray(bq[cs]).reshape(OF, 1),
            "bk": np.ascontiguousarray(bk[cs]).reshape(OF, 1),
            "bv": np.ascontiguousarray(bv[cs]).reshape(OF, 1),
            "wo": np.ascontiguousarray(Wo[cs, :]).astype(BF),
            "mask": mask_host,
        })
    return in_maps


def combine_outputs(results, bo):
    acc = np.zeros((B * S, D), dtype=np.float64)
    for r in results:
        acc += r["out"].astype(np.float64)
    acc += np.asarray(bo, dtype=np.float64)
    return acc.reshape(B, S, D).astype(np.float32)


_NC_CACHE = []


def _get_program():
    if not _NC_CACHE:
        _NC_CACHE.append(build_program())
    return _NC_CACHE[0]


def kernel(**inputs):
    nc = _get_program()
    in_maps = make_in_maps(**inputs)
    res = bass_utils.run_bass_kernel_spmd(
        nc, in_maps, core_ids=list(range(NCORES)))
    return combine_outputs(res.results, inputs["bo"])


# revision 44
# speedup vs baseline: 1.0328x; 1.0314x over previous
"""Multi-head self-attention (B=4, S=2048, D=1024, 16 heads x 64) on 8 TRN2
NeuronCores via Bass/Tile.

Sharding: tensor-parallel over heads. Each core owns 2 heads (128 of the 1024
Q/K/V output features, column-parallel) and the matching 128 rows of Wo
(row-parallel). Every core computes a full-shape partial output; the host sums
the 8 partials (the row-parallel all-reduce) after gathering.

Per-core dataflow (matmul operands bf16, accumulation fp32 in PSUM):
  xT[b]   : [D, S] features-on-partitions (host pre-transposed)
  qT/kT/vT: [128, S]  = (x @ W)^T per core, via lhsT=W k-tiles, rhs=xT
  v_aug   : PE-transpose of vT -> v natural [S,64] per head + ones column
  scoresT : [j, i] per j-tile pair; the two heads occupy row-groups
            (0-63 / 64-127) of the PE array and run CONCURRENTLY (bass
            auto-derives 64x128 row tiling from the base partitions),
            one [128,1024] PSUM tile per jt
  exp     : one ACT Exp op per (i-chunk, j-tile) covering both heads
            (scale=1/8, per-partition bias = attention-mask column)
  PV      : lhsT=[v_h | ones] [128 j, 65], rhs=expT half, accumulated over
            j-tiles -> rows 0-63 ctx^T, row 64 = softmax denominator
  praw    : right after an i-chunk's last PV, a single DVE copy moves
            pc[0:65] PSUM -> SBUF f32, releasing the pc PSUM banks
            immediately (the next i-chunk's PV accumulation reuses them
            without waiting on the normalize chain)
  norm    : reciprocal_approx_fast of praw denom row + bf16 GPSIMD
            partition broadcast, multiply -> ctxT [128, S] bf16; deferred
            into the NEXT i-chunk's first jp iteration so the ACT exp
            stream never idles behind it (pure SBUF, no PSUM coupling)
  out     : lhsT=ctxT tile [128,128], rhs=Wo_c [128,512] chunks; DVE
            PSUM->SBUF bf16 convert; DMA bf16 partial to DRAM via the
            GPSIMD hwdge queue (8 independently-rounded bf16 partials cost
            ~0.01% extra error and halve the output DMA bytes)

The emission is software-pipelined: batch b's attention steps are interleaved
with batch b+1's QKV/V-transpose units and batch b-1's output-projection
units, so the (in-order) PE always has independent matmul work while the
ACT-bound softmax stream runs. x chunk DMAs for batch b+1 are all queued at
the START of attn(b) (xin ring of 6 chunks) so a projection chain never
head-of-line blocks the PE on an in-flight DMA; the sync hwdge queue carries
ONLY the x stream, secondary constants ride the Activation queue, outputs
ride the GPSIMD queue.
"""

import numpy as np
import ml_dtypes

import concourse.bass as bass
import concourse.mybir as mybir
import concourse.tile as tile
from concourse import bacc, bass_utils
from concourse.masks import make_identity

F32 = mybir.dt.float32
BF16 = mybir.dt.bfloat16
AF = mybir.ActivationFunctionType
BF = ml_dtypes.bfloat16
ts = bass.ts

B, S, D = 4, 2048, 1024
NH, HD = 16, 64
NCORES = 8
OF = D // NCORES            # 128 out-features per core (2 heads)
NKT = D // 128              # 8 contraction tiles
NJT = S // 128              # 16 key tiles per batch
NICH = S // 512             # 4 query chunks per batch
NTT = S // 128              # 16 token tiles per batch


def build_program():
    nc = bacc.Bacc("TRN2", target_bir_lowering=False, debug=False,
                   num_devices=NCORES)
    xT_d = nc.dram_tensor("xT", [B, D, S], BF16, kind="ExternalInput")
    wq_d = nc.dram_tensor("wq", [D, OF], BF16, kind="ExternalInput")
    wk_d = nc.dram_tensor("wk", [D, OF], BF16, kind="ExternalInput")
    wv_d = nc.dram_tensor("wv", [D, OF], BF16, kind="ExternalInput")
    bq_d = nc.dram_tensor("bq", [OF, 1], F32, kind="ExternalInput")
    bk_d = nc.dram_tensor("bk", [OF, 1], F32, kind="ExternalInput")
    bv_d = nc.dram_tensor("bv", [OF, 1], F32, kind="ExternalInput")
    wo_d = nc.dram_tensor("wo", [OF, D], BF16, kind="ExternalInput")
    mask_d = nc.dram_tensor("mask", [128, B * NJT], F32, kind="ExternalInput")
    out_d = nc.dram_tensor("out", [B * S, D], BF16, kind="ExternalOutput")

    with tile.TileContext(nc) as tc:
        with (
            tc.tile_pool(name="consts", bufs=1) as consts,
            tc.tile_pool(name="xin", bufs=8) as xin,
            tc.tile_pool(name="qkv", bufs=2) as qkv,
            tc.tile_pool(name="attn", bufs=4) as attn,
            tc.tile_pool(name="ctxp", bufs=2) as ctxp,
            tc.tile_pool(name="outp", bufs=6) as outp,
            tc.tile_pool(name="psum", bufs=2, space="PSUM") as psum,
        ):
            # ---------------- constants ----------------
            ident = consts.tile([128, 128], BF16)
            make_identity(nc, ident)
            # PE warm-up: ~40 no-dependency matmuls on the identity keep
            # the PE HAM activity window busy during the initial x DMA
            # wait, so the real projection chains start at the full
            # 2.4 GHz clock instead of the cold 1.2 GHz default
            warm = psum.tile([128, 1024], F32, tag="sc", name="sc")
            for _ in range(40):
                nc.tensor.matmul(warm[:, 0:128], lhsT=ident, rhs=ident,
                                 start=True, stop=True)
            # preload the ACT Exp table during the DMA wait so the first
            # real softmax op doesn't pay the ~1.3us table load
            warm_act = consts.tile([1, 8], F32, name="warm_act")
            nc.scalar.activation(warm_act, ident[0:1, 0:8], AF.Exp)
            # wq rides the SP queue ahead of the x stream; wk/wv/biases go
            # via the Activation hwdge queue so they don't delay x pieces
            w_sb = {}
            for nm, d, eng in (("q", wq_d, nc.sync), ("k", wk_d, nc.scalar),
                               ("v", wv_d, nc.scalar)):
                t = consts.tile([128, NKT, OF], BF16, name=f"w{nm}_sb")
                eng.dma_start(t, d[:, :].rearrange("(k p) f -> p k f", p=128))
                w_sb[nm] = t
            b_sb = {}
            for nm, d in (("q", bq_d), ("k", bk_d), ("v", bv_d)):
                t = consts.tile([OF, 1], F32, name=f"b{nm}_sb")
                nc.scalar.dma_start(t, d[:, :])
                b_sb[nm] = t
            # mask is needed by the first exp; wo only by the first
            # outproj much later. Both DMAs are emitted after the first
            # QKV chunk (see below), mask first.
            wo_sb = consts.tile([OF, D], BF16)
            mask_sb = consts.tile([128, B * NJT], F32)
            ones_bc = consts.tile([1, 64], BF16, name="ones_bc")
            nc.vector.memset(ones_bc, 1.0)

            state = [dict() for _ in range(B)]

            def xdma_units(b):
                """x chunk DMAs for batch b, split into k-tile pieces so a
                projection chain starts as soon as its first piece lands.
                Queued a full batch ahead of the matmuls that consume them
                (xin ring of 6 chunks)."""
                st = state[b]
                st["xt"] = []
                xr = xT_d[b].rearrange("(k p) t -> p k t", p=128)
                for nch in range(NICH):
                    xt = xin.tile([128, NKT, 512], BF16, name="xt")
                    step = 1 if b == 0 else 2
                    for k0 in range(0, NKT, step):
                        nc.sync.dma_start(
                            xt[:, k0:k0 + step],
                            xr[:, k0:k0 + step, ts(nch, 512)])
                    st["xt"].append(xt)
                    yield

            def qkv_units(b):
                """QKV projections + V transpose for batch b (chunk-paired
                so accumulation chains alternate PSUM banks)."""
                st = state[b]
                pT = {nm: qkv.tile([OF, S], BF16, name=f"{nm}T")
                      for nm in ("q", "k", "v")}
                st["pT"] = pT
                va = [qkv.tile([128, NJT, 65], BF16, name=f"v_aug{h}")
                      for h in range(2)]
                st["va"] = va
                for h in range(2):
                    nc.vector.memset(va[h][:, :, 64:65], 1.0)
                def proj(nm, nch, xt):
                    # two matmuls per yield: finer grains slow the ACT exp
                    # stream ~20% (denser LDW/MM traffic contends with the
                    # activation engine's SBUF/PSUM access)
                    ps = psum.tile([128, 512], F32, tag="mm",
                                   name="ps_qkv")
                    for kt in range(NKT):
                        nc.tensor.matmul(
                            ps, lhsT=w_sb[nm][:, kt, :],
                            rhs=xt[:, kt, :],
                            start=(kt == 0), stop=(kt == NKT - 1),
                        )
                        if kt % 2 == 1:
                            yield
                    nc.vector.tensor_scalar_add(
                        pT[nm][:, ts(nch, 512)], ps, b_sb[nm])

                def vtrans(nch):
                    for jt in range(4 * nch, 4 * nch + 4):
                        pvt = psum.tile([128, 128], BF16, tag="mm",
                                        name="pvt")
                        nc.tensor.transpose(
                            pvt, pT["v"][:, ts(jt, 128)], ident)
                        for h in range(2):
                            nc.vector.tensor_copy(
                                va[h][:, jt, 0:64],
                                pvt[:, h * 64:(h + 1) * 64])
                        yield

                # scores for j-group g need only kT chunk g (plus qT's own
                # chunk), and this batch's attention ich0 runs concurrently
                # with this stream: k/v/vt lead each chunk; q1 follows
                # chunk 1 (ich1 needs it soon after ich0), q2/q3 trail
                yield from proj("k", 0, st["xt"][0])
                yield from proj("q", 0, st["xt"][0])
                yield from proj("v", 0, st["xt"][0])
                yield from vtrans(0)
                yield from proj("k", 1, st["xt"][1])
                yield from proj("v", 1, st["xt"][1])
                yield from vtrans(1)
                yield from proj("k", 2, st["xt"][2])
                yield from proj("q", 1, st["xt"][1])
                yield from proj("v", 2, st["xt"][2])
                yield from vtrans(2)
                yield from proj("k", 3, st["xt"][3])
                yield from proj("v", 3, st["xt"][3])
                yield from vtrans(3)
                yield from proj("q", 2, st["xt"][2])
                yield from proj("q", 3, st["xt"][3])

            pending_den = []  # carried (praw, ctxT, isl) across ichs/batches
            carry_pv = []     # one i-chunk's trailing PVs, flushed after the
                              # NEXT i-chunk's first scores+exp are emitted so
                              # they never head-of-line block those scores

            def flush_carry():
                if not carry_pv:
                    return
                emit_fn, pend, pc, cT, cisl = carry_pv.pop()
                while pend:
                    emit_fn(*pend.pop(0))
                # Evacuate the accumulators (one DVE copy per head):
                # releases the pc PSUM ring for the in-flight i-chunk
                # without waiting on the (deferred) normalize chain.
                praw = [attn.tile([65, 512], F32, name=f"praw{h}",
                                  bufs=3) for h in range(2)]
                for h in range(2):
                    nc.vector.tensor_copy(praw[h], pc[h][0:65, :])
                pending_den.append((praw, cT, cisl))

            def den_units(praw, ctxT, isl):
                """Normalize one i-chunk from its praw SBUF copy: ctx times
                1/denominator. Hoisted into the NEXT i-chunk's first jp
                iteration (right after its first Exp is in flight) so the
                ACT stream never idles behind this DVE/GPSIMD chain. Pure
                SBUF: no PSUM coupling with the PV accumulation."""
                rec16s = []
                for h in range(2):
                    # custom-DVE reciprocal needs a partition-0-aligned
                    # input; stage the denominator row through a [1,512]
                    # tile first
                    den = attn.tile([1, 512], F32, name=f"den{h}")
                    nc.vector.tensor_copy(den, praw[h][64:65, :])
                    rec = attn.tile([1, 512], F32, name=f"rec{h}")
                    nc.vector.reciprocal_approx_fast(rec, den)
                    rec16 = attn.tile([1, 512], BF16, name=f"rec16{h}")
                    nc.vector.tensor_copy(rec16, rec)
                    rec16s.append(rec16)
                    yield
                # both heads' DVE work is queued before the first broadcast
                # so the DVE FIFO never head-of-line blocks on GPSIMD; bf16
                # halves the broadcast bytes (~1us in fp32)
                reps = []
                for h in range(2):
                    rep = attn.tile([64, 512], BF16, name=f"rep{h}")
                    nc.gpsimd.partition_broadcast(rep, rec16s[h])
                    reps.append(rep)
                    yield
                for h in range(2):
                    nc.vector.tensor_mul(
                        ctxT[h * 64:(h + 1) * 64, isl],
                        praw[h][0:64, :], reps[h])
                    yield

            def den_units_pc(pc, ctxT, isl):
                """Normalize the FINAL i-chunk straight from its pc PSUM
                banks (nothing follows, so no praw staging — one less dep
                hop on the kernel-end critical path). Copies and casts ride
                the ACT engine (idle once the exps are done) so the DVE
                queue stays clear for the trailing out-unit evacuations."""
                rec16s = []
                for h in range(2):
                    den = attn.tile([1, 512], F32, name=f"den{h}")
                    nc.scalar.activation(den, pc[h][64:65, :], AF.Copy)
                    rec = attn.tile([1, 512], F32, name=f"rec{h}")
                    nc.vector.reciprocal_approx_fast(rec, den)
                    rec16 = attn.tile([1, 512], BF16, name=f"rec16{h}")
                    nc.vector.tensor_copy(rec16, rec)
                    rec16s.append(rec16)
                    yield
                # K=1 matmul broadcast (ones^T x rec16) on the now-idle PE:
                # ~0.3us vs two serial ~1us GPSIMD partition_broadcasts.
                # ACT (also idle) stages the PSUM result to SBUF — the DVE
                # multiply can read at most one PSUM operand (pc).
                reps = []
                for h in range(2):
                    rep = psum.tile([64, 512], F32, tag="mm",
                                    name=f"repp{h}")
                    nc.tensor.matmul(rep, lhsT=ones_bc, rhs=rec16s[h],
                                     start=True, stop=True)
                    rep_sb = attn.tile([64, 512], BF16, name=f"repsb{h}")
                    nc.scalar.activation(rep_sb, rep, AF.Copy)
                    reps.append(rep_sb)
                    yield
                for h in range(2):
                    nc.vector.tensor_mul(
                        ctxT[h * 64:(h + 1) * 64, isl],
                        pc[h][0:64, :], reps[h])
                    yield

            def attn_units(b):
                """Attention for batch b (normalize chains carried)."""
                st = state[b]
                qT, kT = st["pT"]["q"], st["pT"]["k"]
                va = st["va"]
                ctxT = ctxp.tile([128, S], BF16, name="ctxT")
                st["ctxT"] = ctxT

                for ich in range(NICH):
                    isl = ts(ich, 512)
                    pc = [psum.tile([128, 512], F32, tag="pc", name=f"pc{h}")
                          for h in range(2)]
                    pend_pv = []

                    def emit_pv(jt, et, pc=pc, va=va):
                        for h in range(2):
                            nc.tensor.matmul(
                                pc[h][0:65, :], lhsT=va[h][:, jt, :],
                                rhs=et[:, ts(h, 512)],
                                start=(jt == 0), stop=(jt == NJT - 1),
                            )

                    for jp in range(0, NJT, 2):
                        # two j-tiles of scores back-to-back: their four
                        # row-group-alternating matmuls keep LDWEIGHTS
                        # pull-ahead unblocked (no K=128 matmul between)
                        scs = []
                        for jt in (jp, jp + 1):
                            sc = psum.tile([128, 1024], F32, tag="sc",
                                           name="sc")
                            for h in range(2):
                                hs = slice(h * 64, (h + 1) * 64)
                                nc.tensor.matmul(
                                    sc[:, ts(h, 512)],
                                    lhsT=kT[hs, ts(jt, 128)], rhs=qT[hs, isl],
                                    start=True, stop=True,
                                )
                            scs.append(sc)
                        for idx, jt in enumerate((jp, jp + 1)):
                            et = attn.tile([128, 1024], BF16, name="et",
                                           bufs=8)
                            col = b * NJT + jt
                            nc.scalar.activation(
                                et, scs[idx], AF.Exp,
                                bias=mask_sb[:, col:col + 1], scale=0.125)
                            pend_pv.append((jt, et))
                            if len(pend_pv) > 2:
                                emit_pv(*pend_pv.pop(0))
                            yield
                            if jp == 0 and idx == 0:
                                # the PV flush + praw evac must precede
                                # this i-chunk's first PV (pc ring WAR)
                                flush_carry()
                            if jp == 8 and idx == 0 and pending_den:
                                # normalize chain deferred to mid-chunk:
                                # at jp0 it collides with the batch
                                # boundary's QKV restart on the mm-ring/
                                # DVE queue
                                yield from den_units(*pending_den.pop(0))
                    carry_pv.append((emit_pv, pend_pv, pc, ctxT, isl))

            def final_units():
                """Kernel-end: flush the last i-chunk's PVs and normalize
                straight from PSUM. Run via drain() interleaved with the
                held out-units so their matmuls keep the PE warm while this
                DVE/GPSIMD/ACT chain resolves."""
                emit_fn, pend, pc, cT, cisl = carry_pv.pop()
                while pend:
                    emit_fn(*pend.pop(0))
                yield
                yield from den_units_pc(pc, cT, cisl)
                while pending_den:
                    yield from den_units(*pending_den.pop(0))

            def outproj_units(b):
                """Output projection for batch b. 32 yields."""
                ctxT = state[b]["ctxT"]
                for tt in range(NTT):
                    for oc in range(2):
                        # kernel-end tail: borrow the finished score-PSUM
                        # ring (4 idle banks) so trailing matmuls aren't
                        # paced by the 2-bank mm ring's copy-release rate
                        tg = "sc" if (b == B - 1 and tt >= NTT - 4) else "mm"
                        po = psum.tile([128, 512], F32, tag=tg, name="po")
                        nc.tensor.matmul(
                            po, lhsT=ctxT[:, ts(tt, 128)],
                            rhs=wo_sb[:, ts(oc, 512)],
                            start=True, stop=True,
                        )
                        osb = outp.tile([128, 512], BF16, name="osb")
                        if b == B - 1 and tt >= NTT - 4 and oc == 0:
                            # kernel-end tail: ACT is idle (all Exps done);
                            # route half the PSUM->SBUF copies through it so
                            # the trailing units pipeline 2x
                            nc.scalar.activation(osb, po, AF.Copy)
                        else:
                            nc.vector.tensor_copy(osb, po)
                        # alternate hwdge queues: a single queue issues one
                        # ~650ns trigger at a time and serializes the
                        # kernel-end DMA drain; the last token-tiles split
                        # each piece across BOTH queues
                        rows = slice(b * S + tt * 128, b * S + (tt + 1) * 128)
                        if b == B - 1 and tt >= NTT - 4:
                            nc.gpsimd.dma_start(
                                out_d[rows, oc * 512:oc * 512 + 256],
                                osb[:, 0:256])
                            nc.sync.dma_start(
                                out_d[rows, oc * 512 + 256:oc * 512 + 512],
                                osb[:, 256:512])
                        else:
                            eng = nc.gpsimd if oc == 0 else nc.sync
                            eng.dma_start(out_d[rows, ts(oc, 512)], osb)
                        yield

            def drain(*weighted):
                """weighted: (gen, stride[, delay]) — advance gen every
                `stride` cycles after `delay` cycles. Run until exhausted."""
                live = []
                for w in weighted:
                    g, s, d = (w + (0,)) if len(w) == 2 else w
                    if g is not None:
                        live.append((g, s, d))
                cyc = 0
                while live:
                    nxt = []
                    for g, s, d in live:
                        if cyc >= d and (cyc - d) % s == 0:
                            try:
                                next(g)
                            except StopIteration:
                                continue
                        nxt.append((g, s, d))
                    live = nxt
                    cyc += 1

            def pull(g, n):
                for _ in range(n):
                    try:
                        next(g)
                    except StopIteration:
                        return False
                return True

            g_attn = [attn_units(b) for b in range(B)]
            g_xdma = [xdma_units(b) for b in range(B)]
            g_qkv = [qkv_units(b) for b in range(B)]
            g_out = [outproj_units(b) for b in range(B)]

            # prologue: queue all of batch 0's x stream, then its first
            # QKV chunk; secondary constants (mask before wo — the first
            # exp needs mask) ride the ACT queue behind wk/wv
            pull(g_xdma[0], 4)
            pull(g_qkv[0], 2)
            nc.scalar.dma_start(mask_sb, mask_d[:, :])
            nc.scalar.dma_start(wo_sb, wo_d[:, :])
            pull(g_qkv[0], 10)
            # attention(0) ich0 units unlock per k-chunk: unit u's scores
            # must be emitted AFTER its k-chunk bias-add (emit-before-write
            # would leave the Tile tracker with no dependency to enforce)
            # and its deferred PV after the matching V-transpose. mins[u]
            # is the earliest legal QKV(0)-yield for unit u (+margin).
            mins = [10, 10, 15, 16, 22, 22, 27, 28,
                    34, 34, 43, 44, 50, 50, 55, 56]
            u = cyc = 0
            while pull(g_qkv[0], 1):
                cyc += 1
                if cyc in (40, 44, 48, 52):
                    pull(g_xdma[1], 1)   # x(1) lands before qkv(1) starts
                while u < len(mins) and 12 + cyc >= mins[u] \
                        and pull(g_attn[0], 1):
                    u += 1
            YPI = 22          # attention yields per i-chunk
            for b in range(B - 1):
                # any x(b+1) DMAs not yet queued go first; qkv(b+1) runs
                # 1:1 against attn(b); out(b-1) is delayed so its PE work
                # covers the late (ACT-bound) stretch of attention; x(b+2)
                # queues late in this iteration so its chunks land well
                # before qkv(b+2) chains issue
                pull(g_xdma[b + 1], 4)
                a, q, o = g_attn[b], g_qkv[b + 1], \
                    (g_out[b - 1] if b >= 1 else None)
                cyc = 0
                while pull(a, 1):
                    cyc += 1
                    pull(q, 1)
                    if o is not None and cyc % 3 == 0:
                        pull(o, 1)
                    if b + 2 < B and cyc in (40, 44, 48, 52):
                        pull(g_xdma[b + 2], 1)
                drain((q, 1), (o, 1) if o is not None else (None, 1))
            # final batch: out(3) units become ready 8 per i-chunk; emit them
            # as soon as ready (never earlier — the in-order PE queue would
            # head-of-line block) alongside the tail of out(2)
            a, o2, o3 = g_attn[B - 1], g_out[B - 2], g_out[B - 1]
            cyc = adv3 = 0
            while pull(a, 1):
                cyc += 1
                if (cyc * 2) % 5 < 2:
                    pull(o2, 1)
                # hold the 8 ich2 units (generator positions 17-24) for the
                # final drain: they are dependency-free by then and their
                # matmuls keep the PE busy (HAM warm) while the last
                # i-chunk's normalize chain (DVE/GPSIMD only) runs.
                # -18: ctxT cols for ich i are written by the normalize
                # hoisted at ich i+1's jp8 (cyc ~22i+36); emitting an out
                # unit before its mul would leave the dep untracked
                ready = 8 * max(0, (cyc - 18) // YPI)
                if adv3 < min(ready, 16) and cyc % 2 == 0:
                    if pull(o3, 1):
                        adv3 += 1
            drain((final_units(), 1), (o3, 1))
            drain((o2, 1), (o3, 1))
    nc.finalize()
    return nc


def make_in_maps(x, attention_mask, Wq, bq, Wk, bk, Wv, bv, Wo, bo):
    x = np.asarray(x, dtype=np.float32)
    attention_mask = np.asarray(attention_mask, dtype=np.float32)
    Wq, Wk, Wv, Wo = (np.asarray(a, dtype=np.float32) for a in (Wq, Wk, Wv, Wo))
    bq, bk, bv, bo = (np.asarray(a, dtype=np.float32) for a in (bq, bk, bv, bo))

    xT = np.ascontiguousarray(x.transpose(0, 2, 1)).astype(BF)  # [B, D, S]
    # mask[b,0,0,j] -> [128 partitions, B*NJT] column per (batch, j-tile)
    m = attention_mask.reshape(B, S).reshape(B, NJT, 128)
    mask_host = np.ascontiguousarray(m.transpose(2, 0, 1).reshape(128, B * NJT))

    in_maps = []
    for c in range(NCORES):
        cs = slice(c * OF, (c + 1) * OF)
        in_maps.append({
            "xT": xT,
            "wq": np.ascontiguousarray(Wq[:, cs]).astype(BF),
            "wk": np.ascontiguousarray(Wk[:, cs]).astype(BF),
            "wv": np.ascontiguousarray(Wv[:, cs]).astype(BF),
            "bq": np.ascontiguousarray(bq[cs]).reshape(OF, 1),
            "bk": np.ascontiguousarray(bk[cs]).reshape(OF, 1),
            "bv": np.ascontiguousarray(bv[cs]).reshape(OF, 1),
            "wo": np.ascontiguousarray(Wo[cs, :]).astype(BF),
            "mask": mask_host,
        })
    return in_maps


def combine_outputs(results, bo):
    acc = np.zeros((B * S, D), dtype=np.float64)
    for r in results:
        acc += r["out"].astype(np.float64)
    acc += np.asarray(bo, dtype=np.float64)
    return acc.reshape(B, S, D).astype(np.float32)


_NC_CACHE = []


def _get_program():
    if not _NC_CACHE:
        _NC_CACHE.append(build_program())
    return _NC_CACHE[0]


def kernel(**inputs):
    nc = _get_program()
    in_maps = make_in_maps(**inputs)
    res = bass_utils.run_bass_kernel_spmd(
        nc, in_maps, core_ids=list(range(NCORES)))
    return combine_outputs(res.results, inputs["bo"])


# revision 52
# speedup vs baseline: 1.0514x; 1.0180x over previous
"""Multi-head self-attention (B=4, S=2048, D=1024, 16 heads x 64) on 8 TRN2
NeuronCores via Bass/Tile.

Sharding: tensor-parallel over heads. Each core owns 2 heads (128 of the 1024
Q/K/V output features, column-parallel) and the matching 128 rows of Wo
(row-parallel). Every core computes a full-shape partial output; the host sums
the 8 partials (the row-parallel all-reduce) after gathering.

Per-core dataflow (matmul operands bf16, accumulation fp32 in PSUM):
  xT[b]   : [D, S] features-on-partitions (host pre-transposed)
  qT/kT/vT: [128, S]  = (x @ W)^T per core, via lhsT=W k-tiles, rhs=xT
  v_aug   : PE-transpose of vT -> v natural [S,64] per head + ones column
  scoresT : [j, i] per j-tile pair; the two heads occupy row-groups
            (0-63 / 64-127) of the PE array and run CONCURRENTLY (bass
            auto-derives 64x128 row tiling from the base partitions),
            one [128,1024] PSUM tile per jt
  exp     : one ACT Exp op per (i-chunk, j-tile) covering both heads
            (scale=1/8, per-partition bias = attention-mask column)
  PV      : lhsT=[v_h | ones] [128 j, 65], rhs=expT half, accumulated over
            j-tiles -> rows 0-63 ctx^T, row 64 = softmax denominator
  praw    : right after an i-chunk's last PV, a single DVE copy moves
            pc[0:65] PSUM -> SBUF f32, releasing the pc PSUM banks
            immediately (the next i-chunk's PV accumulation reuses them
            without waiting on the normalize chain)
  norm    : reciprocal_approx_fast of praw denom row + bf16 GPSIMD
            partition broadcast, multiply -> ctxT [128, S] bf16; deferred
            into the NEXT i-chunk's first jp iteration so the ACT exp
            stream never idles behind it (pure SBUF, no PSUM coupling)
  out     : lhsT=ctxT tile [128,128], rhs=Wo_c [128,512] chunks; DVE
            PSUM->SBUF bf16 convert; DMA bf16 partial to DRAM via the
            GPSIMD hwdge queue (8 independently-rounded bf16 partials cost
            ~0.01% extra error and halve the output DMA bytes)

The emission is software-pipelined: batch b's attention steps are interleaved
with batch b+1's QKV/V-transpose units and batch b-1's output-projection
units, so the (in-order) PE always has independent matmul work while the
ACT-bound softmax stream runs. x chunk DMAs for batch b+1 are all queued at
the START of attn(b) (xin ring of 6 chunks) so a projection chain never
head-of-line blocks the PE on an in-flight DMA; the sync hwdge queue carries
ONLY the x stream, secondary constants ride the Activation queue, outputs
ride the GPSIMD queue.
"""

import numpy as np
import ml_dtypes

import concourse.bass as bass
import concourse.mybir as mybir
import concourse.tile as tile
from concourse import bacc, bass_utils
from concourse.masks import make_identity

F32 = mybir.dt.float32
BF16 = mybir.dt.bfloat16
AF = mybir.ActivationFunctionType
BF = ml_dtypes.bfloat16
ts = bass.ts

B, S, D = 4, 2048, 1024
NH, HD = 16, 64
NCORES = 8
OF = D // NCORES            # 128 out-features per core (2 heads)
NKT = D // 128              # 8 contraction tiles
NJT = S // 128              # 16 key tiles per batch
NICH = S // 512             # 4 query chunks per batch
NTT = S // 128              # 16 token tiles per batch


def build_program():
    nc = bacc.Bacc("TRN2", target_bir_lowering=False, debug=False,
                   num_devices=NCORES)
    xT_d = nc.dram_tensor("xT", [B, D, S], BF16, kind="ExternalInput")
    wq_d = nc.dram_tensor("wq", [D, OF], BF16, kind="ExternalInput")
    wk_d = nc.dram_tensor("wk", [D, OF], BF16, kind="ExternalInput")
    wv_d = nc.dram_tensor("wv", [D, OF], BF16, kind="ExternalInput")
    bq_d = nc.dram_tensor("bq", [OF, 1], F32, kind="ExternalInput")
    bk_d = nc.dram_tensor("bk", [OF, 1], F32, kind="ExternalInput")
    bv_d = nc.dram_tensor("bv", [OF, 1], F32, kind="ExternalInput")
    wo_d = nc.dram_tensor("wo", [OF, D], BF16, kind="ExternalInput")
    mask_d = nc.dram_tensor("mask", [128, B * NJT], F32, kind="ExternalInput")
    out_d = nc.dram_tensor("out", [B * S, D], BF16, kind="ExternalOutput")

    with tile.TileContext(nc) as tc:
        with (
            tc.tile_pool(name="consts", bufs=1) as consts,
            tc.tile_pool(name="xin", bufs=8) as xin,
            tc.tile_pool(name="qkv", bufs=2) as qkv,
            tc.tile_pool(name="attn", bufs=4) as attn,
            tc.tile_pool(name="ctxp", bufs=2) as ctxp,
            tc.tile_pool(name="outp", bufs=6) as outp,
            tc.tile_pool(name="psum", bufs=2, space="PSUM") as psum,
        ):
            # ---------------- constants ----------------
            ident = consts.tile([128, 128], BF16)
            make_identity(nc, ident)
            # PE warm-up: ~40 no-dependency matmuls on the identity keep
            # the PE HAM activity window busy during the initial x DMA
            # wait, so the real projection chains start at the full
            # 2.4 GHz clock instead of the cold 1.2 GHz default
            warm = psum.tile([128, 1024], F32, tag="sc", name="sc")
            for _ in range(40):
                nc.tensor.matmul(warm[:, 0:128], lhsT=ident, rhs=ident,
                                 start=True, stop=True)
            # preload the ACT Exp table during the DMA wait so the first
            # real softmax op doesn't pay the ~1.3us table load
            warm_act = consts.tile([1, 8], F32, name="warm_act")
            nc.scalar.activation(warm_act, ident[0:1, 0:8], AF.Exp)
            # wq rides the SP queue ahead of the x stream; wk/wv/biases go
            # via the Activation hwdge queue so they don't delay x pieces
            w_sb = {}
            for nm, d, eng in (("q", wq_d, nc.sync), ("k", wk_d, nc.scalar),
                               ("v", wv_d, nc.scalar)):
                t = consts.tile([128, NKT, OF], BF16, name=f"w{nm}_sb")
                eng.dma_start(t, d[:, :].rearrange("(k p) f -> p k f", p=128))
                w_sb[nm] = t
            b_sb = {}
            for nm, d in (("q", bq_d), ("k", bk_d), ("v", bv_d)):
                t = consts.tile([OF, 1], F32, name=f"b{nm}_sb")
                nc.scalar.dma_start(t, d[:, :])
                b_sb[nm] = t
            # mask is needed by the first exp; wo only by the first
            # outproj much later. Both DMAs are emitted after the first
            # QKV chunk (see below), mask first.
            wo_sb = consts.tile([OF, D], BF16)
            mask_sb = consts.tile([128, B * NJT], F32)
            ones_bc = consts.tile([1, 64], BF16, name="ones_bc")
            nc.vector.memset(ones_bc, 1.0)

            state = [dict() for _ in range(B)]

            def xdma_units(b):
                """x chunk DMAs for batch b, split into k-tile pieces so a
                projection chain starts as soon as its first piece lands.
                Queued a full batch ahead of the matmuls that consume them
                (xin ring of 6 chunks)."""
                st = state[b]
                st["xt"] = []
                xr = xT_d[b].rearrange("(k p) t -> p k t", p=128)
                for nch in range(NICH):
                    xt = xin.tile([128, NKT, 512], BF16, name="xt")
                    step = 1 if b == 0 else 2
                    for k0 in range(0, NKT, step):
                        nc.sync.dma_start(
                            xt[:, k0:k0 + step],
                            xr[:, k0:k0 + step, ts(nch, 512)])
                    st["xt"].append(xt)
                    yield

            def qkv_units(b):
                """QKV projections + V transpose for batch b (chunk-paired
                so accumulation chains alternate PSUM banks)."""
                st = state[b]
                pT = {nm: qkv.tile([OF, S], BF16, name=f"{nm}T")
                      for nm in ("q", "k", "v")}
                st["pT"] = pT
                # both heads' [v | ones] share one tile ([.., 0:65] is
                # head 0, [.., 65:130] head 1) so each V-transpose needs
                # only ONE strided DVE evacuation
                va2 = qkv.tile([128, NJT, 130], BF16, name="v_aug")
                va = [va2[:, :, 0:65], va2[:, :, 65:130]]
                st["va"] = va
                nc.vector.memset(va2[:, :, 64:65], 1.0)
                nc.vector.memset(va2[:, :, 129:130], 1.0)
                def proj(nm, nch, xt):
                    # two matmuls per yield: finer grains slow the ACT exp
                    # stream ~20% (denser LDW/MM traffic contends with the
                    # activation engine's SBUF/PSUM access)
                    ps = psum.tile([128, 512], F32, tag="mm",
                                   name="ps_qkv")
                    for kt in range(NKT):
                        nc.tensor.matmul(
                            ps, lhsT=w_sb[nm][:, kt, :],
                            rhs=xt[:, kt, :],
                            start=(kt == 0), stop=(kt == NKT - 1),
                        )
                        if kt % 2 == 1:
                            yield
                    nc.vector.tensor_scalar_add(
                        pT[nm][:, ts(nch, 512)], ps, b_sb[nm])

                def vtrans(nch):
                    for jt in range(4 * nch, 4 * nch + 4):
                        pvt = psum.tile([128, 128], BF16, tag="mm",
                                        name="pvt")
                        nc.tensor.transpose(
                            pvt, pT["v"][:, ts(jt, 128)], ident)
                        # one strided copy drops both heads' 64 columns
                        # into their va2 slots (skipping the ones columns)
                        nc.vector.tensor_copy(
                            va2[:, jt, :].rearrange(
                                "p (b c) -> p b c", b=2)[:, :, 0:64],
                            pvt[:, :])
                        yield

                # scores for j-group g need only kT chunk g (plus qT's own
                # chunk), and this batch's attention ich0 runs concurrently
                # with this stream: k/v/vt lead each chunk; q1 follows
                # chunk 1 (ich1 needs it soon after ich0), q2/q3 trail
                yield from proj("k", 0, st["xt"][0])
                yield from proj("q", 0, st["xt"][0])
                yield from proj("v", 0, st["xt"][0])
                yield from vtrans(0)
                yield from proj("k", 1, st["xt"][1])
                yield from proj("v", 1, st["xt"][1])
                yield from vtrans(1)
                yield from proj("k", 2, st["xt"][2])
                yield from proj("q", 1, st["xt"][1])
                yield from proj("v", 2, st["xt"][2])
                yield from vtrans(2)
                yield from proj("k", 3, st["xt"][3])
                yield from proj("v", 3, st["xt"][3])
                yield from vtrans(3)
                yield from proj("q", 2, st["xt"][2])
                yield from proj("q", 3, st["xt"][3])

            pending_den = []  # carried (praw, ctxT, isl) across ichs/batches
            carry_pv = []     # one i-chunk's trailing PVs, flushed after the
                              # NEXT i-chunk's first scores+exp are emitted so
                              # they never head-of-line block those scores

            def flush_carry():
                if not carry_pv:
                    return
                emit_fn, pend, pc, cT, cisl = carry_pv.pop()
                while pend:
                    emit_fn(*pend.pop(0))
                # Evacuate the accumulators (one DVE copy per head):
                # releases the pc PSUM ring for the in-flight i-chunk
                # without waiting on the (deferred) normalize chain.
                praw = [attn.tile([65, 512], F32, name=f"praw{h}",
                                  bufs=2) for h in range(2)]
                for h in range(2):
                    nc.vector.tensor_copy(praw[h], pc[h][0:65, :])
                pending_den.append((praw, cT, cisl))

            def den_units(praw, ctxT, isl):
                """Normalize one i-chunk from its praw SBUF copy: ctx times
                1/denominator. Hoisted into the NEXT i-chunk's jp8
                iteration so the ACT stream never idles behind this
                DVE/GPSIMD chain. Both heads ride single double-width DVE
                ops (the ~650ns per-op overhead dominates at these sizes).
                Pure SBUF: no PSUM coupling with the PV accumulation."""
                # custom-DVE reciprocal needs a partition-0-aligned input;
                # stage both denominator rows into one [1,1024] tile
                den = attn.tile([1, 1024], F32, name="den2", bufs=2)
                for h in range(2):
                    nc.vector.tensor_copy(den[:, ts(h, 512)],
                                          praw[h][64:65, :])
                yield
                rec = attn.tile([1, 1024], F32, name="rec2", bufs=2)
                nc.vector.reciprocal_approx_fast(rec, den)
                rec16 = attn.tile([1, 1024], BF16, name="rec162", bufs=2)
                nc.vector.tensor_copy(rec16, rec)
                yield
                rep = attn.tile([64, 1024], BF16, name="rep2", bufs=2)
                nc.gpsimd.partition_broadcast(rep, rec16)
                yield
                for h in range(2):
                    nc.vector.tensor_mul(
                        ctxT[h * 64:(h + 1) * 64, isl],
                        praw[h][0:64, :], rep[:, ts(h, 512)])
                    yield

            def den_units_pc(pc, ctxT, isl):
                """Normalize the FINAL i-chunk straight from its pc PSUM
                banks (nothing follows, so no praw staging — one less dep
                hop on the kernel-end critical path). Copies and casts ride
                the ACT engine (idle once the exps are done) so the DVE
                queue stays clear for the trailing out-unit evacuations."""
                rec16s = []
                for h in range(2):
                    den = attn.tile([1, 512], F32, name=f"den{h}", bufs=1)
                    nc.scalar.activation(den, pc[h][64:65, :], AF.Copy)
                    rec = attn.tile([1, 512], F32, name=f"rec{h}", bufs=1)
                    nc.vector.reciprocal_approx_fast(rec, den)
                    rec16 = attn.tile([1, 512], BF16, name=f"rec16{h}",
                                      bufs=1)
                    nc.vector.tensor_copy(rec16, rec)
                    rec16s.append(rec16)
                    yield
                # K=1 matmul broadcast (ones^T x rec16) on the now-idle PE:
                # ~0.3us vs two serial ~1us GPSIMD partition_broadcasts.
                # ACT (also idle) stages the PSUM result to SBUF — the DVE
                # multiply can read at most one PSUM operand (pc).
                reps = []
                for h in range(2):
                    rep = psum.tile([64, 512], F32, tag="mm",
                                    name=f"repp{h}")
                    nc.tensor.matmul(rep, lhsT=ones_bc, rhs=rec16s[h],
                                     start=True, stop=True)
                    rep_sb = attn.tile([64, 512], BF16, name=f"repsb{h}",
                                       bufs=1)
                    nc.scalar.activation(rep_sb, rep, AF.Copy)
                    reps.append(rep_sb)
                    yield
                for h in range(2):
                    nc.vector.tensor_mul(
                        ctxT[h * 64:(h + 1) * 64, isl],
                        pc[h][0:64, :], reps[h])
                    yield

            def attn_units(b):
                """Attention for batch b (normalize chains carried)."""
                st = state[b]
                qT, kT = st["pT"]["q"], st["pT"]["k"]
                va = st["va"]
                ctxT = ctxp.tile([128, S], BF16, name="ctxT")
                st["ctxT"] = ctxT

                for ich in range(NICH):
                    isl = ts(ich, 512)
                    pc = [psum.tile([128, 512], F32, tag="pc", name=f"pc{h}")
                          for h in range(2)]
                    pend_pv = []

                    def emit_pv(jt, et, pc=pc, va=va):
                        for h in range(2):
                            nc.tensor.matmul(
                                pc[h][0:65, :], lhsT=va[h][:, jt, :],
                                rhs=et[:, ts(h, 512)],
                                start=(jt == 0), stop=(jt == NJT - 1),
                            )

                    for jp in range(0, NJT, 2):
                        # two j-tiles of scores back-to-back: their four
                        # row-group-alternating matmuls keep LDWEIGHTS
                        # pull-ahead unblocked (no K=128 matmul between)
                        scs = []
                        for jt in (jp, jp + 1):
                            sc = psum.tile([128, 1024], F32, tag="sc",
                                           name="sc")
                            for h in range(2):
                                hs = slice(h * 64, (h + 1) * 64)
                                nc.tensor.matmul(
                                    sc[:, ts(h, 512)],
                                    lhsT=kT[hs, ts(jt, 128)], rhs=qT[hs, isl],
                                    start=True, stop=True,
                                )
                            scs.append(sc)
                        for idx, jt in enumerate((jp, jp + 1)):
                            et = attn.tile([128, 1024], BF16, name="et",
                                           bufs=8)
                            col = b * NJT + jt
                            nc.scalar.activation(
                                et, scs[idx], AF.Exp,
                                bias=mask_sb[:, col:col + 1], scale=0.125)
                            pend_pv.append((jt, et))
                            if len(pend_pv) > 2:
                                emit_pv(*pend_pv.pop(0))
                            yield
                            if jp == 0 and idx == 0:
                                # the PV flush + praw evac must precede
                                # this i-chunk's first PV (pc ring WAR)
                                flush_carry()
                            if jp == 8 and idx == 0 and pending_den:
                                # normalize chain deferred to mid-chunk:
                                # at jp0 it collides with the batch
                                # boundary's QKV restart on the mm-ring/
                                # DVE queue
                                yield from den_units(*pending_den.pop(0))
                    carry_pv.append((emit_pv, pend_pv, pc, ctxT, isl))

            def final_units():
                """Kernel-end: flush the last i-chunk's PVs and normalize
                straight from PSUM. Run via drain() interleaved with the
                held out-units so their matmuls keep the PE warm while this
                DVE/GPSIMD/ACT chain resolves."""
                emit_fn, pend, pc, cT, cisl = carry_pv.pop()
                while pend:
                    emit_fn(*pend.pop(0))
                yield
                yield from den_units_pc(pc, cT, cisl)
                while pending_den:
                    yield from den_units(*pending_den.pop(0))

            def outproj_units(b):
                """Output projection for batch b. 32 yields."""
                ctxT = state[b]["ctxT"]
                for tt in range(NTT):
                    for oc in range(2):
                        # kernel-end tail: borrow the finished score-PSUM
                        # ring (4 idle banks) so trailing matmuls aren't
                        # paced by the 2-bank mm ring's copy-release rate
                        tg = "sc" if (b == B - 1 and tt >= NTT - 4) else "mm"
                        po = psum.tile([128, 512], F32, tag=tg, name="po")
                        nc.tensor.matmul(
                            po, lhsT=ctxT[:, ts(tt, 128)],
                            rhs=wo_sb[:, ts(oc, 512)],
                            start=True, stop=True,
                        )
                        osb = outp.tile([128, 512], BF16, name="osb")
                        if b == B - 1 and tt >= NTT - 4 and oc == 0:
                            # kernel-end tail: ACT is idle (all Exps done);
                            # route half the PSUM->SBUF copies through it so
                            # the trailing units pipeline 2x
                            nc.scalar.activation(osb, po, AF.Copy)
                        else:
                            nc.vector.tensor_copy(osb, po)
                        # alternate hwdge queues: a single queue issues one
                        # ~650ns trigger at a time and serializes the
                        # kernel-end DMA drain; the last token-tiles split
                        # each piece across BOTH queues
                        rows = slice(b * S + tt * 128, b * S + (tt + 1) * 128)
                        if b == B - 1 and tt >= NTT - 4:
                            nc.gpsimd.dma_start(
                                out_d[rows, oc * 512:oc * 512 + 256],
                                osb[:, 0:256])
                            nc.sync.dma_start(
                                out_d[rows, oc * 512 + 256:oc * 512 + 512],
                                osb[:, 256:512])
                        else:
                            eng = nc.gpsimd if oc == 0 else nc.sync
                            eng.dma_start(out_d[rows, ts(oc, 512)], osb)
                        yield

            def drain(*weighted):
                """weighted: (gen, stride[, delay]) — advance gen every
                `stride` cycles after `delay` cycles. Run until exhausted."""
                live = []
                for w in weighted:
                    g, s, d = (w + (0,)) if len(w) == 2 else w
                    if g is not None:
                        live.append((g, s, d))
                cyc = 0
                while live:
                    nxt = []
                    for g, s, d in live:
                        if cyc >= d and (cyc - d) % s == 0:
                            try:
                                next(g)
                            except StopIteration:
                                continue
                        nxt.append((g, s, d))
                    live = nxt
                    cyc += 1

            def pull(g, n):
                for _ in range(n):
                    try:
                        next(g)
                    except StopIteration:
                        return False
                return True

            g_attn = [attn_units(b) for b in range(B)]
            g_xdma = [xdma_units(b) for b in range(B)]
            g_qkv = [qkv_units(b) for b in range(B)]
            g_out = [outproj_units(b) for b in range(B)]

            # prologue: queue all of batch 0's x stream, then its first
            # QKV chunk; secondary constants (mask before wo — the first
            # exp needs mask) ride the ACT queue behind wk/wv
            pull(g_xdma[0], 4)
            pull(g_qkv[0], 2)
            nc.scalar.dma_start(mask_sb, mask_d[:, :])
            nc.scalar.dma_start(wo_sb, wo_d[:, :])
            pull(g_qkv[0], 10)
            # attention(0) ich0 units unlock per k-chunk: unit u's scores
            # must be emitted AFTER its k-chunk bias-add (emit-before-write
            # would leave the Tile tracker with no dependency to enforce)
            # and its deferred PV after the matching V-transpose. mins[u]
            # is the earliest legal QKV(0)-yield for unit u (+margin).
            mins = [10, 10, 15, 16, 22, 22, 27, 28,
                    34, 34, 43, 44, 50, 50, 55, 56]
            u = cyc = 0
            while pull(g_qkv[0], 1):
                cyc += 1
                if cyc in (40, 44, 48, 52):
                    pull(g_xdma[1], 1)   # x(1) lands before qkv(1) starts
                while u < len(mins) and 12 + cyc >= mins[u] \
                        and pull(g_attn[0], 1):
                    u += 1
            YPI = 21          # attention yields per i-chunk (16 exp + 5 den)
            for b in range(B - 1):
                # any x(b+1) DMAs not yet queued go first; qkv(b+1) runs
                # 1:1 against attn(b); out(b-1) is delayed so its PE work
                # covers the late (ACT-bound) stretch of attention; x(b+2)
                # queues late in this iteration so its chunks land well
                # before qkv(b+2) chains issue
                pull(g_xdma[b + 1], 4)
                a, q, o = g_attn[b], g_qkv[b + 1], \
                    (g_out[b - 1] if b >= 1 else None)
                cyc = 0
                while pull(a, 1):
                    cyc += 1
                    pull(q, 1)
                    if o is not None and cyc % 3 == 0:
                        pull(o, 1)
                    if b + 2 < B and cyc in (40, 44, 48, 52):
                        pull(g_xdma[b + 2], 1)
                drain((q, 1), (o, 1) if o is not None else (None, 1))
            # final batch: out(3) units become ready 8 per i-chunk; emit them
            # as soon as ready (never earlier — the in-order PE queue would
            # head-of-line block) alongside the tail of out(2)
            a, o2, o3 = g_attn[B - 1], g_out[B - 2], g_out[B - 1]
            cyc = adv3 = 0
            while pull(a, 1):
                cyc += 1
                if (cyc * 2) % 5 < 2:
                    pull(o2, 1)
                # hold the 8 ich2 units (generator positions 17-24) for the
                # final drain: they are dependency-free by then and their
                # matmuls keep the PE busy (HAM warm) while the last
                # i-chunk's normalize chain (DVE/GPSIMD only) runs.
                # -16: ctxT cols for ich i are written by the normalize
                # hoisted at ich i+1's jp8 (muls emitted ~cyc 21i+35);
                # emitting an out unit before its mul would leave the dep
                # untracked (a hardware-only race — sim timing hides it)
                ready = 8 * max(0, (cyc - 16) // YPI)
                if adv3 < min(ready, 16) and cyc % 2 == 0:
                    if pull(o3, 1):
                        adv3 += 1
            drain((final_units(), 1), (o3, 1))
            drain((o2, 1), (o3, 1))
    nc.finalize()
    return nc


def make_in_maps(x, attention_mask, Wq, bq, Wk, bk, Wv, bv, Wo, bo):
    x = np.asarray(x, dtype=np.float32)
    attention_mask = np.asarray(attention_mask, dtype=np.float32)
    Wq, Wk, Wv, Wo = (np.asarray(a, dtype=np.float32) for a in (Wq, Wk, Wv, Wo))
    bq, bk, bv, bo = (np.asarray(a, dtype=np.float32) for a in (bq, bk, bv, bo))

    xT = np.ascontiguousarray(x.transpose(0, 2, 1)).astype(BF)  # [B, D, S]
    # mask[b,0,0,j] -> [128 partitions, B*NJT] column per (batch, j-tile)
    m = attention_mask.reshape(B, S).reshape(B, NJT, 128)
    mask_host = np.ascontiguousarray(m.transpose(2, 0, 1).reshape(128, B * NJT))

    in_maps = []
    for c in range(NCORES):
        cs = slice(c * OF, (c + 1) * OF)
        in_maps.append({
            "xT": xT,
            "wq": np.ascontiguousarray(Wq[:, cs]).astype(BF),
            "wk": np.ascontiguousarray(Wk[:, cs]).astype(BF),
            "wv": np.ascontiguousarray(Wv[:, cs]).astype(BF),
            "bq": np.ascontiguousarray(bq[cs]).reshape(OF, 1),
            "bk": np.ascontiguousarray(bk[cs]).reshape(OF, 1),
            "bv": np.ascontiguousarray(bv[cs]).reshape(OF, 1),
            "wo": np.ascontiguousarray(Wo[cs, :]).astype(BF),
            "mask": mask_host,
        })
    return in_maps


def combine_outputs(results, bo):
    acc = np.zeros((B * S, D), dtype=np.float64)
    for r in results:
        acc += r["out"].astype(np.float64)
    acc += np.asarray(bo, dtype=np.float64)
    return acc.reshape(B, S, D).astype(np.float32)


_NC_CACHE = []


def _get_program():
    if not _NC_CACHE:
        _NC_CACHE.append(build_program())
    return _NC_CACHE[0]


def kernel(**inputs):
    nc = _get_program()
    in_maps = make_in_maps(**inputs)
    res = bass_utils.run_bass_kernel_spmd(
        nc, in_maps, core_ids=list(range(NCORES)))
    return combine_outputs(res.results, inputs["bo"])
